# revision 1
# baseline (speedup 1.0000x reference)
"""Trainium2 Bass kernel for causal self-attention (B=2, S=2048, D=1024, H=16).

Sharding: 8 cores = 2 batch groups x 4 head-groups (tensor parallel).
Core c handles batch b = c // 4 and heads [4*(c%4), 4*(c%4)+4).
Each core computes a partial out-projection [S, D]; the host sums the 4
partials of each batch group (row-parallel TP unshard) and adds bout.

Per-core pipeline (all layouts chosen so no on-device transposes of
activations are needed except small V blocks):
  1. qkvT[col, s] = Wqkv_local.T @ x.T   (x passed pre-transposed, a host
     layout choice; weights are naturally [D, cols] = lhsT layout)
  2. scoresT[k, q] = K_h^T.T @ Q_h per 128-wide k-chunk, causal blocks only.
     Key-padding mask + 1/sqrt(64) scale fold into the ACT exp (per-partition
     bias = per-k bias in this transposed layout).  P = exp(scores') in bf16.
  3. attT[65, q] = V_ext^T @ P  where V_ext = [V_h | ones]: row 64 is the
     softmax denominator.  No separate reduction needed.
  4. normalize per-q: recip = 1/(den + eps) replicated via a K=1 matmul;
     att_n = attT * recip; query-padding mask applied as one big multiply.
  5. out_partial[s, :] = att_n.T @ Wout_local  (att_n is already the lhsT
     layout needed), DMA PSUM -> DRAM directly.
"""

import os
import sys

import numpy as np

for _p in ("/opt/trn_rl_repo",):
    if _p not in sys.path and os.path.isdir(_p):
        sys.path.insert(0, _p)

import concourse.bass as bass
import concourse.mybir as mybir
from concourse import tile
from concourse.bass_utils import run_bass_kernel_spmd

B, S, D, H = 2, 2048, 1024, 16
HD = D // H  # 64
HEADS_PER_CORE = 4
CORES = 8
LOCAL_COLS = 3 * HEADS_PER_CORE * HD  # 768 (q|k|v for 4 heads)
NEG = -1.0e30
EPS = 1.0e-9  # within ACT-reciprocal valid range +-[2^-42, 2^42]

F32 = mybir.dt.float32
F32R = mybir.dt.float32r
BF16 = mybir.dt.bfloat16

AF = mybir.ActivationFunctionType

N_STILE = 4  # 512-wide s tiles
N_KCH = S // 128  # 16 k-chunks
VEXT_W = HEADS_PER_CORE * (HD + 1)  # 260


def round_f32r(a):
    """Round fp32 array to fp32r (11-bit mantissa, round-to-nearest-even)."""
    u = np.ascontiguousarray(a, np.float32).view(np.uint32)
    low = u & np.uint32(0x00000FFF)
    base = u & np.uint32(0xFFFFF000)
    lsb = (u >> np.uint32(12)) & np.uint32(1)
    up = (low > 0x800) | ((low == 0x800) & (lsb == 1))
    return (base + (up.astype(np.uint32) << np.uint32(12))).view(np.float32)



def _split_waits(nc, cap=1):
    """Walrus in this container allows few sync-waits per instruction.
    Hoist excess waits onto preceding same-engine NoOps (same sequencer,
    program order => semantics preserved).  fp32-path Matmult lowers to
    LDW+MM whose LW struct takes no waits at all -> cap 0."""
    uid = [0]
    for fn in nc.m.functions:
        for bb in fn.blocks:
            insts = bb.instructions
            out = []
            for ins in insts:
                icap = 0 if isinstance(ins, mybir.InstMatmult) else cap
                si = ins.sync_info
                waits = list(si.on_wait) if (si and si.on_wait) else []
                if len(waits) > icap:
                    extra = waits[:-icap] if icap else waits
                    keep = waits[-icap:] if icap else []
                    gcap = max(cap, 1)
                    for i in range(0, len(extra), gcap):
                        grp = extra[i : i + gcap]
                        nop = mybir.InstNoOp(
                            name=f"wsplit-{uid[0]}", ins=[], outs=[]
                        )
                        uid[0] += 1
                        nop.engine = ins.engine
                        nop.sync_info = mybir.SyncInfo(on_wait=grp, on_update=[])
                        out.append(nop)
                    si.on_wait = keep
                out.append(ins)
            if len(out) != len(insts):
                insts[:] = out
    return nc


# score-chunk table: per tj (= j//4), list of (start_col, width) chunks of
# the valid q-range [512*tj, 2048), each <= 1024 wide, 512-aligned pieces
CHUNKS = {
    0: [(0, 1024), (1024, 1024)],
    1: [(512, 512), (1024, 1024)],
    2: [(1024, 1024)],
    3: [(1536, 512)],
}


def _chunk_for(tj, col):
    for cs, cw in CHUNKS[tj]:
        if cs <= col < cs + cw:
            return cs, cw
    raise ValueError((tj, col))


def _act_recip(nc, out_ap, in_ap):
    """ACT-engine reciprocal (bass blocks ActivationFunctionType.Reciprocal
    behind an accuracy warning; ~1e-5 rel err is fine for this kernel and it
    replaces a 53us DVE InstReciprocal with one ~2us ACTIVATE)."""
    eng = nc.scalar
    inputs = [eng.lower_ap(in_ap)]
    for v in (0.0, 1.0, 0.0):  # bias, scale, alpha
        inputs.append(mybir.ImmediateValue(dtype=mybir.dt.float32, value=v))
    return eng.add_instruction(
        mybir.InstActivation(
            name=eng.bass.get_next_instruction_name(),
            func=mybir.ActivationFunctionType.Reciprocal,
            ins=inputs,
            outs=[eng.lower_ap(out_ap)],
        )
    )


def build_nc(mm_dt="f32r", p_dt="bf16", split_waits=True):
    """Build the SPMD single-core program (same program on all 8 cores)."""
    nc = bass.Bass()
    mdt = F32R if mm_dt == "f32r" else F32
    pdt = BF16 if p_dt == "bf16" else F32
    scale = float(HD) ** -0.5

    xT = nc.dram_tensor("xT", [D, S], mdt, kind="ExternalInput")
    wqkv = nc.dram_tensor("wqkv", [D, LOCAL_COLS], mdt, kind="ExternalInput")
    bqkv_pc = nc.dram_tensor("bqkv_pc", [128, 6], F32, kind="ExternalInput")
    wout = nc.dram_tensor("wout", [256, D], mdt, kind="ExternalInput")
    kbias = nc.dram_tensor("kbias", [128, N_KCH], F32, kind="ExternalInput")
    qmask_rep = nc.dram_tensor("qmask_rep", [128, S], F32, kind="ExternalInput")
    tri = nc.dram_tensor("tri", [128, 128], F32, kind="ExternalInput")
    ident = nc.dram_tensor("ident", [128, 128], pdt, kind="ExternalInput")
    out = nc.dram_tensor("out", [S, D], F32, kind="ExternalOutput")

    with tile.TileContext(nc) as tc:
        with (
            tc.tile_pool(name="consts", bufs=1) as consts,
            tc.tile_pool(name="persist", bufs=1) as persist,
        ):
            # ---- constants / persistent SBUF ----
            wout_sb = consts.tile([128, 2 * D], mdt)
            for ch in range(2):
                nc.sync.dma_start(
                    wout_sb[:, ch * D : (ch + 1) * D],
                    wout[ch * 128 : (ch + 1) * 128, :],
                )
            kbias_sb = consts.tile([128, N_KCH], F32)
            nc.sync.dma_start(kbias_sb[:], kbias[:])
            qmask_sb = consts.tile([128, S], F32)
            nc.sync.dma_start(qmask_sb[:], qmask_rep[:])
            tri_sb = consts.tile([128, 128], F32)
            nc.sync.dma_start(tri_sb[:], tri[:])

            # qkvT: 6 col-chunks x [128, S] in bf16; 0,1 = q, 2,3 = k, 4,5 = v
            qkvT = persist.tile([128, 6 * S], pdt)
            # V_ext: per k-chunk [128, 260]: 4 heads x (64 V cols + ones col)
            v_ext = persist.tile([128, N_KCH * VEXT_W], pdt)
            # att_u: attended (transposed), unnormalized then normalized in place
            att_u = persist.tile([128, 2 * S], mdt)
            # denominators: one row per head at partition h*32 (engine start-
            # partition constraint: must be 0/32/64/96)
            den4 = persist.tile([128, S], F32)
            recip4 = persist.tile([128, S], F32)

            # ==================== Phase A: QKV ====================
            with (
                tc.tile_pool(name="aconsts", bufs=1) as aconsts,
                tc.tile_pool(name="xs", bufs=3) as xs,
                tc.tile_pool(name="qkv_ps", bufs=6, space="PSUM") as qkv_ps,
                tc.tile_pool(name="tr_ps", bufs=2, space="PSUM") as tr_ps,
            ):
                wqkv_sb = aconsts.tile([128, 8 * LOCAL_COLS], mdt)
                for d in range(8):
                    nc.sync.dma_start(
                        wqkv_sb[:, d * LOCAL_COLS : (d + 1) * LOCAL_COLS],
                        wqkv[d * 128 : (d + 1) * 128, :],
                    )
                bqkv_sb = aconsts.tile([128, 6], F32)
                nc.sync.dma_start(bqkv_sb[:], bqkv_pc[:])
                ident_sb = aconsts.tile([128, 128], pdt)
                nc.sync.dma_start(ident_sb[:], ident[:])
                for t in range(N_STILE):
                    ps = [qkv_ps.tile([128, 512], F32, tag="qkvps", name=f"qkvps_{t}_{i}") for i in range(6)]
                    for d in range(8):
                        xt = xs.tile([128, 512], mdt, tag="xs", name=f"xs_{t}_{d}")
                        nc.gpsimd.dma_start(
                            xt[:], xT[d * 128 : (d + 1) * 128, t * 512 : (t + 1) * 512]
                        )
                        for cc in range(6):
                            nc.tensor.matmul(
                                ps[cc][:],
                                wqkv_sb[:, d * LOCAL_COLS + cc * 128 : d * LOCAL_COLS + (cc + 1) * 128],
                                xt[:],
                                start=(d == 0),
                                stop=(d == 7),
                            )
                    for cc in range(6):
                        nc.vector.tensor_scalar_add(
                            qkvT[:, cc * S + t * 512 : cc * S + (t + 1) * 512],
                            ps[cc][:],
                            bqkv_sb[:, cc : cc + 1],
                        )

                # V transposes: vT chunks 4,5 -> V_ext natural layout (+ones)
                for sc in range(N_KCH):
                    base = sc * VEXT_W
                    nc.any.memset(
                        v_ext[:, base : base + VEXT_W].rearrange(
                            "p (h c) -> p h c", h=HEADS_PER_CORE
                        )[:, :, HD : HD + 1],
                        1.0,
                    )
                    for hp in range(2):  # head pairs
                        tp = tr_ps.tile([128, 128], pdt, tag="trps", name=f"trps_{sc}_{hp}")
                        nc.tensor.transpose(
                            tp[:],
                            qkvT[:, (4 + hp) * S + sc * 128 : (4 + hp) * S + (sc + 1) * 128],
                            ident_sb[:],
                        )
                        nc.vector.tensor_copy(
                            v_ext[:, base + hp * 2 * (HD + 1) : base + (hp * 2 + 2) * (HD + 1)]
                            .rearrange("p (h c) -> p h c", h=2)[:, :, 0:HD],
                            tp[:].rearrange("p (h c) -> p h c", h=2),
                        )

            # ==================== Phase B: attention ====================
            with (
                tc.tile_pool(name="sc_ps", bufs=3, space="PSUM") as sc_ps,
                tc.tile_pool(name="av_ps", bufs=2, space="PSUM") as av_ps,
                tc.tile_pool(name="pt", bufs=4) as ptp,
                tc.tile_pool(name="rr", bufs=3) as rrp,
                tc.tile_pool(name="outsb", bufs=2) as outsb,
                tc.tile_pool(name="dram", bufs=1, space="DRAM") as dramp,
            ):
                recip4_dram = dramp.tile([4, S], F32, name="recip4_dram")
                def emit_scores_pair(p, j):
                    qch = p
                    kch = 2 + p
                    tj = j // 4
                    for ci, (cs, cw) in enumerate(CHUNKS[tj]):
                        tiles = []
                        for hh in range(2):
                            h = 2 * p + hh
                            qrow = hh * 64
                            sps = sc_ps.tile(
                                [128, 1024], F32, tag="scps", name=f"scps_{h}_{j}_{ci}"
                            )
                            tiles.append(sps)
                        # alternate heads per 512-slice: adjacent matmuls use
                        # disjoint row groups -> concurrent execution
                        for o in range(0, cw, 512):
                            t = (cs + o) // 512
                            for hh in range(2):
                                qrow = hh * 64
                                nc.tensor.matmul(
                                    tiles[hh][:, o : o + 512],
                                    qkvT[qrow : qrow + 64, kch * S + j * 128 : kch * S + (j + 1) * 128],
                                    qkvT[qrow : qrow + 64, qch * S + t * 512 : qch * S + (t + 1) * 512],
                                    start=True,
                                    stop=True,
                                )
                        for hh in range(2):
                            h = 2 * p + hh
                            sps = tiles[hh]
                            pt = ptp.tile(
                                [128, cw], pdt, tag=f"pt{cw}",
                                bufs=(32 if cw == 1024 else 16),
                                name=f"pt_{h}_{j}_{ci}",
                            )
                            if ci == 0:
                                db = j * 128 - cs  # diag block offset in chunk
                                nc.vector.tensor_add(
                                    sps[:, db : db + 128], sps[:, db : db + 128], tri_sb[:]
                                )
                                if db > 0:
                                    nc.any.memset(pt[:, 0:db], 0.0)
                                nc.scalar.activation(
                                    pt[:, db:cw], sps[:, db:cw], AF.Exp,
                                    bias=kbias_sb[:, j : j + 1], scale=scale,
                                )
                            else:
                                nc.scalar.activation(
                                    pt[:, 0:cw], sps[:, 0:cw], AF.Exp,
                                    bias=kbias_sb[:, j : j + 1], scale=scale,
                                )
                            pts[(h, j, cs)] = pt

                def emit_av_pair(p, t):
                    qch = p
                    jmax = 4 * t + 3
                    for hh in range(2):
                        h = 2 * p + hh
                        qrow = hh * 64
                        aps = av_ps.tile(
                            [65, 512], F32, tag="avps", padded_shape=[128, 512],
                            name=f"avps_{h}_{t}",
                        )
                        for j in range(jmax + 1):
                            tj = j // 4
                            cs, cw = _chunk_for(tj, t * 512)
                            off = t * 512 - cs
                            nc.tensor.matmul(
                                aps[:],
                                v_ext[:, j * VEXT_W + h * (HD + 1) : j * VEXT_W + (h + 1) * (HD + 1)],
                                pts[(h, j, cs)][:, off : off + 512],
                                start=(j == 0),
                                stop=(j == jmax),
                            )
                        nc.vector.tensor_scalar_add(
                            den4[h * 32 : h * 32 + 1, t * 512 : (t + 1) * 512],
                            aps[64:65, :],
                            EPS,
                        )
                        nc.scalar.activation(
                            att_u[qrow : qrow + 64, qch * S + t * 512 : qch * S + (t + 1) * 512],
                            aps[0:64, :],
                            AF.Identity,
                        )

                pts = {}

                def emit_norm_outproj(t):
                    """All 4 heads' denominators for q-tile t are ready:
                    reciprocal + qmask fold + broadcast + normalize + project."""
                    for h in range(HEADS_PER_CORE):
                        _act_recip(
                            nc,
                            recip4[h * 32 : h * 32 + 1, t * 512 : (t + 1) * 512],
                            den4[h * 32 : h * 32 + 1, t * 512 : (t + 1) * 512],
                        )
                        nc.vector.tensor_mul(
                            recip4[h * 32 : h * 32 + 1, t * 512 : (t + 1) * 512],
                            recip4[h * 32 : h * 32 + 1, t * 512 : (t + 1) * 512],
                            qmask_sb[h * 32 : h * 32 + 1, t * 512 : (t + 1) * 512],
                        )
                    nc.sync.dma_start(
                        recip4_dram[:, t * 512 : (t + 1) * 512],
                        recip4[:, t * 512 : (t + 1) * 512]
                        .rearrange("(a b) c -> a b c", b=32)[:, 0:1, :]
                        .rearrange("a b c -> (a b) c"),
                    )
                    for qch in range(2):
                        rr = rrp.tile([128, 512], F32, tag="rr", name=f"rr_{qch}_{t}")
                        for hh in range(2):
                            h = qch * 2 + hh
                            nc.sync.dma_start(
                                rr[hh * 64 : (hh + 1) * 64, :],
                                recip4_dram[h : h + 1, t * 512 : (t + 1) * 512].to_broadcast((64, 512)),
                            )
                        sl = att_u[:, qch * S + t * 512 : qch * S + (t + 1) * 512]
                        nc.vector.tensor_mul(sl, sl, rr[:])
                    for st in range(4 * t, 4 * t + 4):
                        for n in range(2):
                            ops = av_ps.tile([128, 512], F32, tag="avps", name=f"outps_{st}_{n}")
                            for ch in range(2):
                                nc.tensor.matmul(
                                    ops[:],
                                    att_u[:, ch * S + st * 128 : ch * S + (st + 1) * 128],
                                    wout_sb[:, ch * D + n * 512 : ch * D + (n + 1) * 512],
                                    start=(ch == 0),
                                    stop=(ch == 1),
                                )
                            osb = outsb.tile([128, 512], F32, tag="outsb", name=f"outsb_{st}_{n}")
                            nc.vector.tensor_copy(osb[:], ops[:])
                            nc.sync.dma_start(
                                out[st * 128 : (st + 1) * 128, n * 512 : (n + 1) * 512],
                                osb[:],
                            )

                # head-PAIR emission with per-tile AV drains; once pair 1's
                # AV(t) lands, all four heads of q-tile t are complete ->
                # normalize + out-project t while pair 1 continues scoring.
                for p in range(2):
                    for j in range(N_KCH):
                        emit_scores_pair(p, j)
                        if j % 4 == 3:
                            t = j // 4
                            emit_av_pair(p, t)
                            if p == 1:
                                emit_norm_outproj(t)

    return _split_waits(nc) if split_waits else nc


def make_in_maps(x, attention_mask, Wqkv, bqkv, Wout, mm_dt="f32r"):
    """Shard full inputs into the 8 per-core input dicts."""
    rnd = round_f32r if mm_dt == "f32r" else (lambda a: np.ascontiguousarray(a, np.float32))
    x = np.asarray(x, np.float32)
    attention_mask = np.asarray(attention_mask)
    Wqkv = np.asarray(Wqkv, np.float32)
    bqkv = np.asarray(bqkv, np.float32)
    Wout = np.asarray(Wout, np.float32)

    import ml_dtypes

    tri = np.where(
        np.arange(128)[:, None] <= np.arange(128)[None, :], 0.0, NEG
    ).astype(np.float32)
    ident = np.eye(128, dtype=ml_dtypes.bfloat16)

    in_maps = []
    for c in range(CORES):
        b, g = divmod(c, 4)
        cs = 256 * g  # local col start within each of q/k/v blocks
        wq = Wqkv[:, cs : cs + 256]
        wk = Wqkv[:, D + cs : D + cs + 256]
        wv = Wqkv[:, 2 * D + cs : 2 * D + cs + 256]
        w_local = np.ascontiguousarray(np.concatenate([wq, wk, wv], axis=1))
        b_local = np.concatenate(
            [bqkv[cs : cs + 256], bqkv[D + cs : D + cs + 256], bqkv[2 * D + cs : 2 * D + cs + 256]]
        )
        bqkv_pc = np.ascontiguousarray(b_local.reshape(6, 128).T)
        wout_l = np.ascontiguousarray(Wout[cs : cs + 256, :])
        m = attention_mask[b].astype(np.float32)
        kb = np.where(m > 0, 0.0, NEG).astype(np.float32)
        kbias_pc = np.ascontiguousarray(kb.reshape(N_KCH, 128).T)
        qmask_rep = np.ascontiguousarray(np.broadcast_to(m[None, :], (128, S)))
        in_maps.append(
            {
                "xT": rnd(x[b].T),
                "wqkv": rnd(w_local),
                "bqkv_pc": bqkv_pc,
                "wout": rnd(wout_l),
                "kbias": kbias_pc,
                "qmask_rep": qmask_rep,
                "tri": tri,
                "ident": ident,
            }
        )
    return in_maps


_NC_CACHE = {}


def _get_nc(mm_dt="f32r", p_dt="bf16"):
    key = (mm_dt, p_dt)
    if key not in _NC_CACHE:
        _NC_CACHE[key] = build_nc(*key)
    return _NC_CACHE[key]


def kernel(x, attention_mask, Wqkv, bqkv, Wout, bout, _trace=False, _trace_kwargs=None):
    bout = np.asarray(bout, np.float32)
    mm_dt = os.environ.get("ATTN_MM_DT", "f32r")
    p_dt = os.environ.get("ATTN_P_DT", "bf16")
    in_maps = make_in_maps(x, attention_mask, Wqkv, bqkv, Wout, mm_dt=mm_dt)
    nc = _get_nc(mm_dt, p_dt)
    res = run_bass_kernel_spmd(
        nc,
        in_maps,
        list(range(CORES)),
        trace=_trace,
        **(_trace_kwargs or {}),
    )
    outs = [res.results[c]["out"] for c in range(CORES)]
    full = np.empty((B, S, D), np.float32)
    for b in range(B):
        full[b] = outs[4 * b] + outs[4 * b + 1] + outs[4 * b + 2] + outs[4 * b + 3] + bout
    if _trace:
        return full, res
    return full



# revision 8
# speedup vs baseline: 1.0906x; 1.0906x over previous
"""Trainium2 Bass kernel for causal self-attention (B=2, S=2048, D=1024, H=16).

Sharding: 8 cores = 2 batch groups x 4 head-groups (tensor parallel).
Core c handles batch b = c // 4 and heads [4*(c%4), 4*(c%4)+4).
Each core computes a partial out-projection [S, D] in bf16; the host sums the
4 partials of each batch group (row-parallel TP unshard) and adds bout.

v2 design (wavefront): all dtypes bf16 on device (fp32 PSUM accumulate).
One wavefront per 512-wide q-tile t:
  1. qkvT[:, t] = Wqkv_local.T @ x.T in 6 single-psum-bank rounds (cc), with
     per-d-chunk weight tiles so the first matmul starts as soon as chunk 0
     and the first x tile land.
  2. scores for q-tile t against all k-chunks j <= 4t+3, exact-start pieces
     (stream [max(512t,128j), 512(t+1)) only).  P = exp(scale*s + kbias) into
     a 64-slot SBUF arena, slot (h, j), rewritten every wavefront.
  3. AV chunks interleaved with scores per head (lag 2) so the ACT engine
     (EXP) keeps pace with the PE; V_ext rows come from DMA-transposes
     (XBAR) of the v chunks - no PE transposes.
  4. denominators via the ones-column of V_ext; one batched ACT reciprocal
     per tile (recip table preloaded by a dummy op), qmask folded in with one
     DVE multiply, SBUF->SBUF broadcast DMA, normalize, out-project.  The
     norm chain of tile t is hidden under wavefront t+1's QKV rounds.
"""

import os
import sys

import numpy as np

for _p in ("/opt/trn_rl_repo",):
    if _p not in sys.path and os.path.isdir(_p):
        sys.path.insert(0, _p)

import concourse.bass as bass
import concourse.mybir as mybir
from concourse import tile
from concourse.bass_utils import run_bass_kernel_spmd

B, S, D, H = 2, 2048, 1024, 16
HD = D // H  # 64
HEADS_PER_CORE = 4
CORES = 8
LOCAL_COLS = 3 * HEADS_PER_CORE * HD  # 768 (q|k|v for 4 heads)
NEG = -1.0e30
EPS = 1.0e-9

F32 = mybir.dt.float32
BF16 = mybir.dt.bfloat16

AF = mybir.ActivationFunctionType

N_KCH = S // 128  # 16 k-chunks
VEXT_W = HEADS_PER_CORE * (HD + 1)  # 260
SCALE = float(HD) ** -0.5


def _split_waits(nc, cap=1):
    """Walrus in this container allows few sync-waits per instruction.
    Hoist excess waits onto preceding same-engine NoOps (same sequencer,
    program order => semantics preserved).  fp32-path Matmult lowers to
    LDW+MM whose LW struct takes no waits at all -> cap 0."""
    uid = [0]
    for fn in nc.m.functions:
        for bb in fn.blocks:
            insts = bb.instructions
            out = []
            for ins in insts:
                icap = 0 if isinstance(ins, mybir.InstMatmult) else cap
                si = ins.sync_info
                waits = list(si.on_wait) if (si and si.on_wait) else []
                if len(waits) > icap:
                    extra = waits[:-icap] if icap else waits
                    keep = waits[-icap:] if icap else []
                    gcap = max(cap, 1)
                    for i in range(0, len(extra), gcap):
                        grp = extra[i : i + gcap]
                        nop = mybir.InstNoOp(
                            name=f"wsplit-{uid[0]}", ins=[], outs=[]
                        )
                        uid[0] += 1
                        nop.engine = ins.engine
                        nop.sync_info = mybir.SyncInfo(on_wait=grp, on_update=[])
                        out.append(nop)
                    si.on_wait = keep
                out.append(ins)
            if len(out) != len(insts):
                insts[:] = out
    return nc


def _act_recip(nc, out_ap, in_ap, bias=0.0):
    """ACT-engine reciprocal (bass blocks ActivationFunctionType.Reciprocal
    behind an accuracy warning; ~1e-5 rel err is fine for this kernel).
    Computes 1/(x + bias)."""
    eng = nc.scalar
    inputs = [eng.lower_ap(in_ap)]
    for v in (bias, 1.0, 0.0):  # bias, scale, alpha
        inputs.append(mybir.ImmediateValue(dtype=mybir.dt.float32, value=v))
    return eng.add_instruction(
        mybir.InstActivation(
            name=eng.bass.get_next_instruction_name(),
            func=mybir.ActivationFunctionType.Reciprocal,
            ins=inputs,
            outs=[eng.lower_ap(out_ap)],
        )
    )


def build_nc(split_waits=True, debug=False):
    """Build the SPMD single-core program (same program on all 8 cores)."""
    nc = bass.Bass()

    xT = nc.dram_tensor("xT", [D, S], BF16, kind="ExternalInput")
    wqkv = nc.dram_tensor("wqkv", [D, LOCAL_COLS], BF16, kind="ExternalInput")
    bqkv_pc = nc.dram_tensor("bqkv_pc", [128, 6], F32, kind="ExternalInput")
    wout = nc.dram_tensor("wout", [256, D], BF16, kind="ExternalInput")
    kbias = nc.dram_tensor("kbias", [128, N_KCH], F32, kind="ExternalInput")
    qmask_rep = nc.dram_tensor("qmask_rep", [128, S], BF16, kind="ExternalInput")
    tri = nc.dram_tensor("tri", [128, 128], F32, kind="ExternalInput")
    out = nc.dram_tensor("out", [S, D], BF16, kind="ExternalOutput")
    if debug:
        dbg_qkvT = nc.dram_tensor("dbg_qkvT", [128, 6 * S], BF16, kind="ExternalOutput")
        dbg_vext = nc.dram_tensor("dbg_vext", [128, N_KCH * VEXT_W], BF16, kind="ExternalOutput")
        dbg_att = nc.dram_tensor("dbg_att", [128, 2 * S], BF16, kind="ExternalOutput")
        dbg_den = nc.dram_tensor("dbg_den", [128, S], F32, kind="ExternalOutput")
        dbg_arena = nc.dram_tensor("dbg_arena", [128, 64 * 512], BF16, kind="ExternalOutput")

    with tile.TileContext(nc) as tc:
        with (
            tc.tile_pool(name="consts", bufs=1) as consts,
            tc.tile_pool(name="persist", bufs=1) as persist,
            tc.tile_pool(name="xs", bufs=16) as xs,
            tc.tile_pool(name="rr", bufs=2) as rrp,
            tc.tile_pool(name="osb", bufs=3) as osbp,
            tc.tile_pool(name="vtr", bufs=4) as vtrp,
            tc.tile_pool(name="qkv_ps", bufs=2, space="PSUM") as qkv_ps,
            tc.tile_pool(name="sc_ps", bufs=3, space="PSUM") as sc_ps,
            tc.tile_pool(name="avout", bufs=3, space="PSUM") as avout,
            tc.tile_pool(name="dram", bufs=1, space="DRAM") as dramp,
        ):
            recip_dram = dramp.tile([4, S], BF16, name="recip_dram")
            # ---- constants ----
            wq_sb = []
            for d in range(8):
                w = consts.tile([128, LOCAL_COLS], BF16, name=f"wq_{d}")
                nc.sync.dma_start(w[:], wqkv[d * 128 : (d + 1) * 128, :])
                wq_sb.append(w)
            wout_sb = consts.tile([128, 2 * D], BF16)
            for ch in range(2):
                nc.sync.dma_start(
                    wout_sb[:, ch * D : (ch + 1) * D],
                    wout[ch * 128 : (ch + 1) * 128, :],
                )
            bqkv_sb = consts.tile([128, 6], F32)
            nc.sync.dma_start(bqkv_sb[:], bqkv_pc[:])
            kbias_sb = consts.tile([128, N_KCH], F32)
            nc.sync.dma_start(kbias_sb[:], kbias[:])
            qmask_sb = consts.tile([128, S], BF16)
            nc.sync.dma_start(qmask_sb[:], qmask_rep[:])
            tri_sb = consts.tile([128, 128], F32)
            nc.sync.dma_start(tri_sb[:], tri[:])

            # ---- persistent state ----
            qkvT = persist.tile([128, 6 * S], BF16)
            v_ext = persist.tile([128, N_KCH * VEXT_W], BF16)
            att_u = persist.tile([128, 2 * S], BF16)
            den4 = persist.tile([128, S], F32)
            recip4 = persist.tile([128, S], BF16)
            arena = persist.tile([128, 64 * 512], BF16)
            scratch = persist.tile([1, 8], F32)

            def slot(h, j):
                return (h * N_KCH + j) * 512

            xts = {}

            def emit_qkv_round(t, cc):
                ps = qkv_ps.tile([128, 512], F32, tag="qkvps", name=f"qps_{t}_{cc}")
                for d in range(8):
                    nc.tensor.matmul(
                        ps[:],
                        wq_sb[d][:, cc * 128 : (cc + 1) * 128],
                        xts[(t, d)][:],
                        start=(d == 0),
                        stop=(d == 7),
                    )
                nc.vector.tensor_scalar_add(
                    qkvT[:, cc * S + t * 512 : cc * S + (t + 1) * 512],
                    ps[:],
                    bqkv_sb[:, cc : cc + 1],
                )

            def emit_score(t, p, hh, j):
                h = 2 * p + hh
                qrow = hh * 64
                db = max(0, j * 128 - t * 512)
                sps = sc_ps.tile([128, 512], F32, tag="scps", name=f"sps_{t}_{h}_{j}")
                nc.tensor.matmul(
                    sps[:, db:512],
                    qkvT[qrow : qrow + 64, (2 + p) * S + j * 128 : (2 + p) * S + (j + 1) * 128],
                    qkvT[qrow : qrow + 64, p * S + t * 512 + db : p * S + (t + 1) * 512],
                    start=True,
                    stop=True,
                )
                if j >= 4 * t:
                    nc.vector.tensor_add(
                        sps[:, db : db + 128], sps[:, db : db + 128], tri_sb[:]
                    )
                    if db > 0:
                        nc.gpsimd.memset(arena[:, slot(h, j) : slot(h, j) + db], 0.0)
                nc.scalar.activation(
                    arena[:, slot(h, j) + db : slot(h, j) + 512],
                    sps[:, db:512],
                    AF.Exp,
                    bias=kbias_sb[:, j : j + 1],
                    scale=SCALE,
                )

            def emit_av_chunk(t, h, j, aps, last):
                nc.tensor.matmul(
                    aps[:],
                    v_ext[:, j * VEXT_W + h * (HD + 1) : j * VEXT_W + (h + 1) * (HD + 1)],
                    arena[:, slot(h, j) : slot(h, j) + 512],
                    start=(j == 0),
                    stop=last,
                )

            def emit_head_tail(t, p, hh, aps):
                h = 2 * p + hh
                qrow = hh * 64
                nc.vector.tensor_copy(
                    den4[32 * h : 32 * h + 1, t * 512 : (t + 1) * 512], aps[64:65, :]
                )
                nc.vector.tensor_copy(
                    att_u[qrow : qrow + 64, p * S + t * 512 : p * S + (t + 1) * 512],
                    aps[0:64, :],
                )

            rr_tiles = {}

            def emit_norm(t):
                _act_recip(
                    nc,
                    recip4[:, t * 512 : (t + 1) * 512],
                    den4[:, t * 512 : (t + 1) * 512],
                    bias=EPS,
                )
                nc.vector.tensor_mul(
                    recip4[:, t * 512 : (t + 1) * 512],
                    recip4[:, t * 512 : (t + 1) * 512],
                    qmask_sb[:, t * 512 : (t + 1) * 512],
                )
                nc.sync.dma_start(
                    recip_dram[:, t * 512 : (t + 1) * 512],
                    recip4[:, t * 512 : (t + 1) * 512]
                    .rearrange("(a b) c -> a b c", b=32)[:, 0:1, :]
                    .rearrange("a b c -> (a b) c"),
                )
                for p in range(2):
                    rr = rrp.tile([128, 512], BF16, tag="rr", name=f"rr_{t}_{p}")
                    for hh in range(2):
                        h = 2 * p + hh
                        nc.sync.dma_start(
                            rr[hh * 64 : (hh + 1) * 64, :],
                            recip_dram[h : h + 1, t * 512 : (t + 1) * 512]
                            .to_broadcast((64, 512)),
                        )
                    rr_tiles[(t, p)] = rr

            def emit_outproj(t):
                for p in range(2):
                    rr = rr_tiles.pop((t, p))
                    nc.vector.tensor_mul(
                        att_u[:, p * S + t * 512 : p * S + (t + 1) * 512],
                        att_u[:, p * S + t * 512 : p * S + (t + 1) * 512],
                        rr[:],
                    )
                for st in range(4 * t, 4 * t + 4):
                    for n in range(2):
                        ops = avout.tile(
                            [128, 512], F32, tag="avps", name=f"ops_{st}_{n}"
                        )
                        for ch in range(2):
                            nc.tensor.matmul(
                                ops[:],
                                att_u[:, ch * S + st * 128 : ch * S + (st + 1) * 128],
                                wout_sb[:, ch * D + n * 512 : ch * D + (n + 1) * 512],
                                start=(ch == 0),
                                stop=(ch == 1),
                            )
                        ob = osbp.tile([128, 512], BF16, tag="osb", name=f"ob_{st}_{n}")
                        nc.vector.tensor_copy(ob[:], ops[:])
                        nc.sync.dma_start(
                            out[st * 128 : (st + 1) * 128, n * 512 : (n + 1) * 512],
                            ob[:],
                        )

            # ==================== main wavefront loop ====================
            first = True
            for t in range(4):
                n_j = 4 * t + 4
                for d in range(8):
                    xt = xs.tile([128, 512], BF16, tag="xs", name=f"xs_{t}_{d}")
                    nc.gpsimd.dma_start(
                        xt[:], xT[d * 128 : (d + 1) * 128, t * 512 : (t + 1) * 512]
                    )
                    xts[(t, d)] = xt
                if first:
                    # engine-side memsets after the first DMA triggers
                    nc.gpsimd.memset(den4[:], 1.0)
                    nc.gpsimd.memset(scratch[:], 1.0)
                    first = False

                emit_qkv_round(t, 0)
                emit_qkv_round(t, 1)
                if t > 0:
                    emit_outproj(t - 1)

                # head 0: non-diag scores (use k from previous wavefronts)
                for j in range(0, 4 * t):
                    emit_score(t, 0, 0, j)
                emit_qkv_round(t, 2)
                emit_qkv_round(t, 3)

                aps0 = avout.tile(
                    [65, 512], F32, tag="avps", padded_shape=[128, 512],
                    name=f"aps_{t}_0",
                )
                for j in range(4 * t, n_j):
                    emit_score(t, 0, 0, j)
                for j in range(0, 4 * t):
                    emit_av_chunk(t, 0, j, aps0, last=False)

                emit_qkv_round(t, 4)
                emit_qkv_round(t, 5)
                # V transposes for this wavefront's k-chunks: XBAR DMA into
                # 32B-aligned scratch, then DVE copy into the 65-col layout
                for sc in range(4 * t, n_j):
                    base = sc * VEXT_W
                    nc.gpsimd.memset(
                        v_ext[:, base : base + VEXT_W].rearrange(
                            "p (h c) -> p h c", h=HEADS_PER_CORE
                        )[:, :, HD : HD + 1],
                        1.0,
                    )
                    for hp in range(2):
                        vt = vtrp.tile([128, 128], BF16, tag="vtr", name=f"vt_{sc}_{hp}")
                        nc.sync.dma_start_transpose(
                            vt[:],
                            qkvT[:, (4 + hp) * S + sc * 128 : (4 + hp) * S + (sc + 1) * 128],
                        )
                        nc.vector.tensor_copy(
                            v_ext[:, base + hp * 130 : base + hp * 130 + 130]
                            .rearrange("p (g c) -> p g c", g=2)[:, :, 0:HD],
                            vt[:].rearrange("p (g c) -> p g c", g=2),
                        )

                for j in range(4 * t, n_j):
                    emit_av_chunk(t, 0, j, aps0, last=(j == n_j - 1))
                emit_head_tail(t, 0, 0, aps0)

                # heads 1..3: scores/AV interleaved (AV lags scores by 2)
                for p, hh in ((0, 1), (1, 0), (1, 1)):
                    h = 2 * p + hh
                    aps = avout.tile(
                        [65, 512], F32, tag="avps", padded_shape=[128, 512],
                        name=f"aps_{t}_{h}",
                    )
                    for j in range(n_j):
                        emit_score(t, p, hh, j)
                        if j >= 2:
                            emit_av_chunk(t, h, j - 2, aps, last=False)
                    emit_av_chunk(t, h, n_j - 2, aps, last=False)
                    emit_av_chunk(t, h, n_j - 1, aps, last=True)
                    emit_head_tail(t, p, hh, aps)

                # preload the reciprocal ACT table off the critical path
                _act_recip(nc, scratch[0:1, 0:1], scratch[0:1, 4:5], bias=EPS)
                if debug and t == 3:
                    # dump arena before norm (holds wavefront-3 P pieces)
                    nc.sync.dma_start(dbg_arena[:], arena[:])
                    nc.sync.dma_start(dbg_att[:], att_u[:])
                emit_norm(t)
            emit_outproj(3)
            if debug:
                nc.sync.dma_start(dbg_qkvT[:], qkvT[:])
                nc.sync.dma_start(dbg_vext[:], v_ext[:])
                nc.sync.dma_start(dbg_den[:], den4[:])

    return _split_waits(nc) if split_waits else nc


def make_in_maps(x, attention_mask, Wqkv, bqkv, Wout):
    """Shard full inputs into the 8 per-core input dicts."""
    import ml_dtypes

    BF = ml_dtypes.bfloat16
    x = np.asarray(x, np.float32)
    attention_mask = np.asarray(attention_mask)
    Wqkv = np.asarray(Wqkv, np.float32)
    bqkv = np.asarray(bqkv, np.float32)
    Wout = np.asarray(Wout, np.float32)

    tri = np.where(
        np.arange(128)[:, None] <= np.arange(128)[None, :], 0.0, NEG
    ).astype(np.float32)

    in_maps = []
    for c in range(CORES):
        b, g = divmod(c, 4)
        cs = 256 * g  # local col start within each of q/k/v blocks
        wq = Wqkv[:, cs : cs + 256]
        wk = Wqkv[:, D + cs : D + cs + 256]
        wv = Wqkv[:, 2 * D + cs : 2 * D + cs + 256]
        w_local = np.ascontiguousarray(np.concatenate([wq, wk, wv], axis=1))
        b_local = np.concatenate(
            [bqkv[cs : cs + 256], bqkv[D + cs : D + cs + 256], bqkv[2 * D + cs : 2 * D + cs + 256]]
        )
        bqkv_pc = np.ascontiguousarray(b_local.reshape(6, 128).T)
        wout_l = np.ascontiguousarray(Wout[cs : cs + 256, :])
        m = attention_mask[b].astype(np.float32)
        kb = np.where(m > 0, 0.0, NEG).astype(np.float32)
        kbias_pc = np.ascontiguousarray(kb.reshape(N_KCH, 128).T)
        qmask_rep = np.ascontiguousarray(
            np.broadcast_to(m[None, :], (128, S)).astype(BF)
        )
        in_maps.append(
            {
                "xT": np.ascontiguousarray(x[b].T).astype(BF),
                "wqkv": w_local.astype(BF),
                "bqkv_pc": bqkv_pc,
                "wout": wout_l.astype(BF),
                "kbias": kbias_pc,
                "qmask_rep": qmask_rep,
                "tri": tri,
            }
        )
    return in_maps


_NC_CACHE = {}


def _get_nc():
    if "nc" not in _NC_CACHE:
        _NC_CACHE["nc"] = build_nc()
    return _NC_CACHE["nc"]


def kernel(x, attention_mask, Wqkv, bqkv, Wout, bout, _trace=False, _trace_kwargs=None):
    bout = np.asarray(bout, np.float32)
    in_maps = make_in_maps(x, attention_mask, Wqkv, bqkv, Wout)
    nc = _get_nc()
    res = run_bass_kernel_spmd(
        nc,
        in_maps,
        list(range(CORES)),
        trace=_trace,
        **(_trace_kwargs or {}),
    )
    outs = [np.asarray(res.results[c]["out"], np.float32) for c in range(CORES)]
    full = np.empty((B, S, D), np.float32)
    for b in range(B):
        full[b] = outs[4 * b] + outs[4 * b + 1] + outs[4 * b + 2] + outs[4 * b + 3] + bout
    if _trace:
        return full, res
    return full


# revision 12
# speedup vs baseline: 1.2425x; 1.1393x over previous
"""Trainium2 Bass kernel for causal self-attention (B=2, S=2048, D=1024, H=16).

Sharding: 8 cores = 2 batch groups x 4 head-groups (tensor parallel).
Core c handles batch b = c // 4 and heads [4*(c%4), 4*(c%4)+4).
Each core computes a partial out-projection [S, D] in bf16; the host sums the
4 partials of each batch group (row-parallel TP unshard) and adds bout.

v2 design (wavefront): all dtypes bf16 on device (fp32 PSUM accumulate).
One wavefront per 512-wide q-tile t:
  1. qkvT[:, t] = Wqkv_local.T @ x.T in 6 single-psum-bank rounds (cc), with
     per-d-chunk weight tiles so the first matmul starts as soon as chunk 0
     and the first x tile land.
  2. scores for q-tile t against all k-chunks j <= 4t+3, exact-start pieces
     (stream [max(512t,128j), 512(t+1)) only).  P = exp(scale*s + kbias) into
     a 64-slot SBUF arena, slot (h, j), rewritten every wavefront.
  3. AV chunks interleaved with scores per head (lag 2) so the ACT engine
     (EXP) keeps pace with the PE; V_ext rows come from DMA-transposes
     (XBAR) of the v chunks - no PE transposes.
  4. denominators via the ones-column of V_ext; one batched ACT reciprocal
     per tile (recip table preloaded by a dummy op), qmask folded in with one
     DVE multiply, SBUF->SBUF broadcast DMA, normalize, out-project.  The
     norm chain of tile t is hidden under wavefront t+1's QKV rounds.
"""

import os
import sys

import numpy as np

for _p in ("/opt/trn_rl_repo",):
    if _p not in sys.path and os.path.isdir(_p):
        sys.path.insert(0, _p)

import concourse.bass as bass
import concourse.mybir as mybir
from concourse import tile
from concourse.bass_utils import run_bass_kernel_spmd

B, S, D, H = 2, 2048, 1024, 16
HD = D // H  # 64
HEADS_PER_CORE = 4
CORES = 8
LOCAL_COLS = 3 * HEADS_PER_CORE * HD  # 768 (q|k|v for 4 heads)
NEG = -1.0e30
EPS = 1.0e-9

F32 = mybir.dt.float32
BF16 = mybir.dt.bfloat16

AF = mybir.ActivationFunctionType

N_KCH = S // 128  # 16 k-chunks
VEXT_W = HEADS_PER_CORE * (HD + 1)  # 260
SCALE = float(HD) ** -0.5


def _split_waits(nc, cap=1):
    """Walrus in this container allows few sync-waits per instruction.
    Hoist excess waits onto preceding same-engine NoOps (same sequencer,
    program order => semantics preserved).  fp32-path Matmult lowers to
    LDW+MM whose LW struct takes no waits at all -> cap 0."""
    uid = [0]
    for fn in nc.m.functions:
        for bb in fn.blocks:
            insts = bb.instructions
            out = []
            for ins in insts:
                icap = 0 if isinstance(ins, mybir.InstMatmult) else cap
                si = ins.sync_info
                waits = list(si.on_wait) if (si and si.on_wait) else []
                if len(waits) > icap:
                    extra = waits[:-icap] if icap else waits
                    keep = waits[-icap:] if icap else []
                    gcap = max(cap, 1)
                    for i in range(0, len(extra), gcap):
                        grp = extra[i : i + gcap]
                        nop = mybir.InstNoOp(
                            name=f"wsplit-{uid[0]}", ins=[], outs=[]
                        )
                        uid[0] += 1
                        nop.engine = ins.engine
                        nop.sync_info = mybir.SyncInfo(on_wait=grp, on_update=[])
                        out.append(nop)
                    si.on_wait = keep
                out.append(ins)
            if len(out) != len(insts):
                insts[:] = out
    return nc


def _act_recip(nc, out_ap, in_ap, bias=0.0):
    """ACT-engine reciprocal (bass blocks ActivationFunctionType.Reciprocal
    behind an accuracy warning; ~1e-5 rel err is fine for this kernel).
    Computes 1/(x + bias)."""
    eng = nc.scalar
    inputs = [eng.lower_ap(in_ap)]
    for v in (bias, 1.0, 0.0):  # bias, scale, alpha
        inputs.append(mybir.ImmediateValue(dtype=mybir.dt.float32, value=v))
    return eng.add_instruction(
        mybir.InstActivation(
            name=eng.bass.get_next_instruction_name(),
            func=mybir.ActivationFunctionType.Reciprocal,
            ins=inputs,
            outs=[eng.lower_ap(out_ap)],
        )
    )


def build_nc(split_waits=True, debug=False):
    """Build the SPMD single-core program (same program on all 8 cores)."""
    nc = bass.Bass()

    xT = nc.dram_tensor("xT", [D, S], BF16, kind="ExternalInput")
    wqkv = nc.dram_tensor("wqkv", [D, LOCAL_COLS], BF16, kind="ExternalInput")
    bqkv_pc = nc.dram_tensor("bqkv_pc", [128, 6], F32, kind="ExternalInput")
    wout = nc.dram_tensor("wout", [256, D], BF16, kind="ExternalInput")
    kbias = nc.dram_tensor("kbias", [128, N_KCH], F32, kind="ExternalInput")
    qmask_rep = nc.dram_tensor("qmask_rep", [128, S], BF16, kind="ExternalInput")
    tri = nc.dram_tensor("tri", [128, 128], F32, kind="ExternalInput")
    out = nc.dram_tensor("out", [S, D], BF16, kind="ExternalOutput")
    if debug:
        dbg_qkvT = nc.dram_tensor("dbg_qkvT", [128, 6 * S], BF16, kind="ExternalOutput")
        dbg_vext = nc.dram_tensor("dbg_vext", [128, N_KCH * VEXT_W], BF16, kind="ExternalOutput")
        dbg_att = nc.dram_tensor("dbg_att", [128, 2 * S], BF16, kind="ExternalOutput")
        dbg_den = nc.dram_tensor("dbg_den", [128, S], F32, kind="ExternalOutput")
        dbg_arena = nc.dram_tensor("dbg_arena", [128, 64 * 512], BF16, kind="ExternalOutput")

    with tile.TileContext(nc) as tc:
        with (
            tc.tile_pool(name="consts", bufs=1) as consts,
            tc.tile_pool(name="persist", bufs=1) as persist,
            tc.tile_pool(name="xs", bufs=16) as xs,
            tc.tile_pool(name="rr", bufs=2) as rrp,
            tc.tile_pool(name="osb", bufs=3) as osbp,
            tc.tile_pool(name="vtr", bufs=4) as vtrp,
            tc.tile_pool(name="qkv_ps", bufs=2, space="PSUM") as qkv_ps,
            tc.tile_pool(name="sc_ps", bufs=2, space="PSUM") as sc_ps,
            tc.tile_pool(name="avout", bufs=2, space="PSUM") as avout,
            tc.tile_pool(name="dram", bufs=1, space="DRAM") as dramp,
        ):
            recip_dram = dramp.tile([4, S], BF16, name="recip_dram")
            # ---- constants ----
            wq_sb = []
            for d in range(8):
                w = consts.tile([128, LOCAL_COLS], BF16, name=f"wq_{d}")
                nc.sync.dma_start(w[:], wqkv[d * 128 : (d + 1) * 128, :])
                wq_sb.append(w)
            wout_sb = consts.tile([128, 2 * D], BF16)
            for ch in range(2):
                nc.sync.dma_start(
                    wout_sb[:, ch * D : (ch + 1) * D],
                    wout[ch * 128 : (ch + 1) * 128, :],
                )
            bqkv_sb = consts.tile([128, 6], F32)
            nc.sync.dma_start(bqkv_sb[:], bqkv_pc[:])
            kbias_sb = consts.tile([128, N_KCH], F32)
            nc.sync.dma_start(kbias_sb[:], kbias[:])
            qmask_sb = consts.tile([128, S], BF16)
            nc.sync.dma_start(qmask_sb[:], qmask_rep[:])
            tri_sb = consts.tile([128, 128], F32)
            nc.sync.dma_start(tri_sb[:], tri[:])

            # ---- persistent state ----
            qkvT = persist.tile([128, 6 * S], BF16)
            v_ext = persist.tile([128, N_KCH * VEXT_W], BF16)
            att_u = persist.tile([128, 2 * S], BF16)
            den4 = persist.tile([128, S], F32)
            recip4 = persist.tile([128, S], BF16)
            arena = persist.tile([128, 64 * 512], BF16)
            scratch = persist.tile([1, 8], F32)

            def slot(h, j):
                # heads of a pair adjacent so one EXP writes both halves
                return (j * HEADS_PER_CORE + h) * 512

            xts = {}

            def emit_qkv_round(t, cc):
                ps = qkv_ps.tile([128, 512], F32, tag="qkvps", name=f"qps_{t}_{cc}")
                for d in range(8):
                    nc.tensor.matmul(
                        ps[:],
                        wq_sb[d][:, cc * 128 : (cc + 1) * 128],
                        xts[(t, d)][:],
                        start=(d == 0),
                        stop=(d == 7),
                    )
                nc.vector.tensor_scalar_add(
                    qkvT[:, cc * S + t * 512 : cc * S + (t + 1) * 512],
                    ps[:],
                    bqkv_sb[:, cc : cc + 1],
                )

            def emit_score_pair(t, p, j):
                """Scores piece for both heads of pair p vs k-chunk j, q-tile t.
                Two matmuls into one [128,1024] psum (one 512-half per head),
                a single EXP (same per-key bias) into two adjacent arena slots."""
                db = max(0, j * 128 - t * 512)
                sps = sc_ps.tile([128, 1024], F32, tag="scps", name=f"sps_{t}_{p}_{j}")
                for hh in range(2):
                    qrow = hh * 64
                    nc.tensor.matmul(
                        sps[:, hh * 512 + db : hh * 512 + 512],
                        qkvT[qrow : qrow + 64, (2 + p) * S + j * 128 : (2 + p) * S + (j + 1) * 128],
                        qkvT[qrow : qrow + 64, p * S + t * 512 + db : p * S + (t + 1) * 512],
                        start=True,
                        stop=True,
                    )
                s0 = slot(2 * p, j)
                if j >= 4 * t:
                    for hh in range(2):
                        nc.vector.tensor_add(
                            sps[:, hh * 512 + db : hh * 512 + db + 128],
                            sps[:, hh * 512 + db : hh * 512 + db + 128],
                            tri_sb[:],
                        )
                    if db > 0:
                        nc.gpsimd.memset(
                            arena[:, s0 : s0 + 1024].rearrange(
                                "p (g c) -> p g c", g=2
                            )[:, :, 0:db],
                            0.0,
                        )
                nc.scalar.activation(
                    arena[:, s0 : s0 + 1024].rearrange("p (g c) -> p g c", g=2)[
                        :, :, db:512
                    ],
                    sps[:].rearrange("p (g c) -> p g c", g=2)[:, :, db:512],
                    AF.Exp,
                    bias=kbias_sb[:, j : j + 1],
                    scale=SCALE,
                )

            def emit_av_chunk(t, h, j, aps, last):
                nc.tensor.matmul(
                    aps[:],
                    v_ext[:, j * VEXT_W + h * (HD + 1) : j * VEXT_W + (h + 1) * (HD + 1)],
                    arena[:, slot(h, j) : slot(h, j) + 512],
                    start=(j == 0),
                    stop=last,
                )

            def emit_head_tail(t, p, hh, aps):
                h = 2 * p + hh
                qrow = hh * 64
                nc.vector.tensor_copy(
                    den4[32 * h : 32 * h + 1, t * 512 : (t + 1) * 512], aps[64:65, :]
                )
                nc.vector.tensor_copy(
                    att_u[qrow : qrow + 64, p * S + t * 512 : p * S + (t + 1) * 512],
                    aps[0:64, :],
                )

            rr_tiles = {}

            def emit_norm(t):
                _act_recip(
                    nc,
                    recip4[:, t * 512 : (t + 1) * 512],
                    den4[:, t * 512 : (t + 1) * 512],
                    bias=EPS,
                )
                nc.vector.tensor_mul(
                    recip4[:, t * 512 : (t + 1) * 512],
                    recip4[:, t * 512 : (t + 1) * 512],
                    qmask_sb[:, t * 512 : (t + 1) * 512],
                )
                nc.sync.dma_start(
                    recip_dram[:, t * 512 : (t + 1) * 512],
                    recip4[:, t * 512 : (t + 1) * 512]
                    .rearrange("(a b) c -> a b c", b=32)[:, 0:1, :]
                    .rearrange("a b c -> (a b) c"),
                )
                for p in range(2):
                    rr = rrp.tile([128, 512], BF16, tag="rr", name=f"rr_{t}_{p}")
                    for hh in range(2):
                        h = 2 * p + hh
                        nc.sync.dma_start(
                            rr[hh * 64 : (hh + 1) * 64, :],
                            recip_dram[h : h + 1, t * 512 : (t + 1) * 512]
                            .to_broadcast((64, 512)),
                        )
                    rr_tiles[(t, p)] = rr

            def emit_outproj(t):
                for p in range(2):
                    rr = rr_tiles.pop((t, p))
                    nc.vector.tensor_mul(
                        att_u[:, p * S + t * 512 : p * S + (t + 1) * 512],
                        att_u[:, p * S + t * 512 : p * S + (t + 1) * 512],
                        rr[:],
                    )
                for st in range(4 * t, 4 * t + 4):
                    for n in range(2):
                        ops = avout.tile(
                            [128, 512], F32, tag="avps", name=f"ops_{st}_{n}"
                        )
                        for ch in range(2):
                            nc.tensor.matmul(
                                ops[:],
                                att_u[:, ch * S + st * 128 : ch * S + (st + 1) * 128],
                                wout_sb[:, ch * D + n * 512 : ch * D + (n + 1) * 512],
                                start=(ch == 0),
                                stop=(ch == 1),
                            )
                        ob = osbp.tile([128, 512], BF16, tag="osb", name=f"ob_{st}_{n}")
                        nc.vector.tensor_copy(ob[:], ops[:])
                        nc.sync.dma_start(
                            out[st * 128 : (st + 1) * 128, n * 512 : (n + 1) * 512],
                            ob[:],
                        )

            # ==================== main wavefront loop ====================
            first = True
            for t in range(4):
                n_j = 4 * t + 4
                for d in range(8):
                    xt = xs.tile([128, 512], BF16, tag="xs", name=f"xs_{t}_{d}")
                    nc.gpsimd.dma_start(
                        xt[:], xT[d * 128 : (d + 1) * 128, t * 512 : (t + 1) * 512]
                    )
                    xts[(t, d)] = xt
                if first:
                    # engine-side memsets after the first DMA triggers
                    nc.gpsimd.memset(den4[:], 1.0)
                    nc.gpsimd.memset(scratch[:], 1.0)
                    first = False

                emit_qkv_round(t, 0)
                emit_qkv_round(t, 1)
                if t > 0:
                    emit_outproj(t - 1)
                emit_qkv_round(t, 4)
                emit_qkv_round(t, 5)
                # V transposes early: XBAR DMA into 32B-aligned scratch, then
                # DVE copy into the 65-col layout; sync engine runs these
                # while the PE chews on scores
                for sc in range(4 * t, n_j):
                    base = sc * VEXT_W
                    nc.gpsimd.memset(
                        v_ext[:, base : base + VEXT_W].rearrange(
                            "p (h c) -> p h c", h=HEADS_PER_CORE
                        )[:, :, HD : HD + 1],
                        1.0,
                    )
                    for hp in range(2):
                        vt = vtrp.tile([128, 128], BF16, tag="vtr", name=f"vt_{sc}_{hp}")
                        nc.sync.dma_start_transpose(
                            vt[:],
                            qkvT[:, (4 + hp) * S + sc * 128 : (4 + hp) * S + (sc + 1) * 128],
                        )
                        nc.vector.tensor_copy(
                            v_ext[:, base + hp * 130 : base + hp * 130 + 130]
                            .rearrange("p (g c) -> p g c", g=2)[:, :, 0:HD],
                            vt[:].rearrange("p (g c) -> p g c", g=2),
                        )

                # per pair: scores (paired-head pieces) interleaved with AV
                # chunks (lag 2); diag scores need k(t) so qkv rounds 2,3 are
                # emitted just before pair 0 reaches them
                for p in range(2):
                    aps = [
                        avout.tile(
                            [65, 512], F32, tag="avps", padded_shape=[128, 512],
                            name=f"aps_{t}_{2 * p + hh}",
                        )
                        for hh in range(2)
                    ]

                    def avpair(j, last):
                        for hh in range(2):
                            emit_av_chunk(t, 2 * p + hh, j, aps[hh], last)

                    for j in range(0, 4 * t):
                        emit_score_pair(t, p, j)
                        if j >= 2:
                            avpair(j - 2, False)
                    if p == 0:
                        emit_qkv_round(t, 2)
                        emit_qkv_round(t, 3)
                    for j in range(4 * t, n_j):
                        emit_score_pair(t, p, j)
                        if j >= 2:
                            avpair(j - 2, False)
                    avpair(n_j - 2, False)
                    avpair(n_j - 1, True)
                    for hh in range(2):
                        emit_head_tail(t, p, hh, aps[hh])

                # preload the reciprocal ACT table off the critical path
                _act_recip(nc, scratch[0:1, 0:1], scratch[0:1, 4:5], bias=EPS)
                if debug and t == 3:
                    # dump arena before norm (holds wavefront-3 P pieces)
                    nc.sync.dma_start(dbg_arena[:], arena[:])
                    nc.sync.dma_start(dbg_att[:], att_u[:])
                emit_norm(t)
            emit_outproj(3)
            if debug:
                nc.sync.dma_start(dbg_qkvT[:], qkvT[:])
                nc.sync.dma_start(dbg_vext[:], v_ext[:])
                nc.sync.dma_start(dbg_den[:], den4[:])

    return _split_waits(nc) if split_waits else nc


def make_in_maps(x, attention_mask, Wqkv, bqkv, Wout):
    """Shard full inputs into the 8 per-core input dicts."""
    import ml_dtypes

    BF = ml_dtypes.bfloat16
    x = np.asarray(x, np.float32)
    attention_mask = np.asarray(attention_mask)
    Wqkv = np.asarray(Wqkv, np.float32)
    bqkv = np.asarray(bqkv, np.float32)
    Wout = np.asarray(Wout, np.float32)

    tri = np.where(
        np.arange(128)[:, None] <= np.arange(128)[None, :], 0.0, NEG
    ).astype(np.float32)

    in_maps = []
    for c in range(CORES):
        b, g = divmod(c, 4)
        cs = 256 * g  # local col start within each of q/k/v blocks
        wq = Wqkv[:, cs : cs + 256]
        wk = Wqkv[:, D + cs : D + cs + 256]
        wv = Wqkv[:, 2 * D + cs : 2 * D + cs + 256]
        w_local = np.ascontiguousarray(np.concatenate([wq, wk, wv], axis=1))
        b_local = np.concatenate(
            [bqkv[cs : cs + 256], bqkv[D + cs : D + cs + 256], bqkv[2 * D + cs : 2 * D + cs + 256]]
        )
        bqkv_pc = np.ascontiguousarray(b_local.reshape(6, 128).T)
        wout_l = np.ascontiguousarray(Wout[cs : cs + 256, :])
        m = attention_mask[b].astype(np.float32)
        kb = np.where(m > 0, 0.0, NEG).astype(np.float32)
        kbias_pc = np.ascontiguousarray(kb.reshape(N_KCH, 128).T)
        qmask_rep = np.ascontiguousarray(
            np.broadcast_to(m[None, :], (128, S)).astype(BF)
        )
        in_maps.append(
            {
                "xT": np.ascontiguousarray(x[b].T).astype(BF),
                "wqkv": w_local.astype(BF),
                "bqkv_pc": bqkv_pc,
                "wout": wout_l.astype(BF),
                "kbias": kbias_pc,
                "qmask_rep": qmask_rep,
                "tri": tri,
            }
        )
    return in_maps


_NC_CACHE = {}


def _get_nc():
    if "nc" not in _NC_CACHE:
        _NC_CACHE["nc"] = build_nc()
    return _NC_CACHE["nc"]


def kernel(x, attention_mask, Wqkv, bqkv, Wout, bout, _trace=False, _trace_kwargs=None):
    bout = np.asarray(bout, np.float32)
    in_maps = make_in_maps(x, attention_mask, Wqkv, bqkv, Wout)
    nc = _get_nc()
    res = run_bass_kernel_spmd(
        nc,
        in_maps,
        list(range(CORES)),
        trace=_trace,
        **(_trace_kwargs or {}),
    )
    outs = [np.asarray(res.results[c]["out"], np.float32) for c in range(CORES)]
    full = np.empty((B, S, D), np.float32)
    for b in range(B):
        full[b] = outs[4 * b] + outs[4 * b + 1] + outs[4 * b + 2] + outs[4 * b + 3] + bout
    if _trace:
        return full, res
    return full


# revision 21
# speedup vs baseline: 1.3699x; 1.1025x over previous
"""Trainium2 Bass kernel for causal self-attention (B=2, S=2048, D=1024, H=16).

Sharding: 8 cores = 2 batch groups x 4 head-groups (tensor parallel).
Core c handles batch b = c // 4 and heads [4*(c%4), 4*(c%4)+4).
Each core computes a partial out-projection [S, D] in bf16; the host sums the
4 partials of each batch group (row-parallel TP unshard) and adds bout.

v2 design (wavefront): all dtypes bf16 on device (fp32 PSUM accumulate).
One wavefront per 512-wide q-tile t:
  1. qkvT[:, t] = Wqkv_local.T @ x.T in 6 single-psum-bank rounds (cc), with
     per-d-chunk weight tiles so the first matmul starts as soon as chunk 0
     and the first x tile land.
  2. scores for q-tile t against all k-chunks j <= 4t+3, exact-start pieces
     (stream [max(512t,128j), 512(t+1)) only).  P = exp(scale*s + kbias) into
     a 64-slot SBUF arena, slot (h, j), rewritten every wavefront.
  3. AV chunks interleaved with scores per head (lag 2) so the ACT engine
     (EXP) keeps pace with the PE; V_ext rows come from DMA-transposes
     (XBAR) of the v chunks - no PE transposes.
  4. denominators via the ones-column of V_ext; one batched ACT reciprocal
     per tile (recip table preloaded by a dummy op), qmask folded in with one
     DVE multiply, SBUF->SBUF broadcast DMA, normalize, out-project.  The
     norm chain of tile t is hidden under wavefront t+1's QKV rounds.
"""

import os
import sys

import numpy as np

for _p in ("/opt/trn_rl_repo",):
    if _p not in sys.path and os.path.isdir(_p):
        sys.path.insert(0, _p)

import concourse.bass as bass
import concourse.mybir as mybir
from concourse import tile
from concourse.bass_utils import run_bass_kernel_spmd

B, S, D, H = 2, 2048, 1024, 16
HD = D // H  # 64
HEADS_PER_CORE = 4
CORES = 8
LOCAL_COLS = 3 * HEADS_PER_CORE * HD  # 768 (q|k|v for 4 heads)
NEG = -1.0e30
EPS = 1.0e-9

F32 = mybir.dt.float32
BF16 = mybir.dt.bfloat16

AF = mybir.ActivationFunctionType

N_KCH = S // 128  # 16 k-chunks
VEXT_W = HEADS_PER_CORE * (HD + 1)  # 260
SCALE = float(HD) ** -0.5


def _split_waits(nc, cap=1):
    """Walrus in this container allows few sync-waits per instruction.
    Hoist excess waits onto preceding same-engine NoOps (same sequencer,
    program order => semantics preserved).  fp32-path Matmult lowers to
    LDW+MM whose LW struct takes no waits at all -> cap 0."""
    uid = [0]
    for fn in nc.m.functions:
        for bb in fn.blocks:
            insts = bb.instructions
            out = []
            for ins in insts:
                icap = 0 if isinstance(ins, mybir.InstMatmult) else cap
                si = ins.sync_info
                waits = list(si.on_wait) if (si and si.on_wait) else []
                if len(waits) > icap:
                    extra = waits[:-icap] if icap else waits
                    keep = waits[-icap:] if icap else []
                    gcap = max(cap, 1)
                    for i in range(0, len(extra), gcap):
                        grp = extra[i : i + gcap]
                        nop = mybir.InstNoOp(
                            name=f"wsplit-{uid[0]}", ins=[], outs=[]
                        )
                        uid[0] += 1
                        nop.engine = ins.engine
                        nop.sync_info = mybir.SyncInfo(on_wait=grp, on_update=[])
                        out.append(nop)
                    si.on_wait = keep
                out.append(ins)
            if len(out) != len(insts):
                insts[:] = out
    return nc


def _act_recip(nc, out_ap, in_ap, bias=0.0):
    """ACT-engine reciprocal (bass blocks ActivationFunctionType.Reciprocal
    behind an accuracy warning; ~1e-5 rel err is fine for this kernel).
    Computes 1/(x + bias)."""
    eng = nc.scalar
    inputs = [eng.lower_ap(in_ap)]
    for v in (bias, 1.0, 0.0):  # bias, scale, alpha
        inputs.append(mybir.ImmediateValue(dtype=mybir.dt.float32, value=v))
    return eng.add_instruction(
        mybir.InstActivation(
            name=eng.bass.get_next_instruction_name(),
            func=mybir.ActivationFunctionType.Reciprocal,
            ins=inputs,
            outs=[eng.lower_ap(out_ap)],
        )
    )


def build_nc(split_waits=True, debug=False):
    """Build the SPMD single-core program (same program on all 8 cores)."""
    nc = bass.Bass()

    xT = nc.dram_tensor("xT", [D, S], BF16, kind="ExternalInput")
    wqkv = nc.dram_tensor("wqkv", [D, LOCAL_COLS], BF16, kind="ExternalInput")
    bqkv_pc = nc.dram_tensor("bqkv_pc", [128, 6], F32, kind="ExternalInput")
    wout = nc.dram_tensor("wout", [256, D], BF16, kind="ExternalInput")
    kbias = nc.dram_tensor("kbias", [128, N_KCH], F32, kind="ExternalInput")
    qmask_rep = nc.dram_tensor("qmask_rep", [128, S], BF16, kind="ExternalInput")
    tri = nc.dram_tensor("tri", [128, 128], F32, kind="ExternalInput")
    out = nc.dram_tensor("out", [S, D], BF16, kind="ExternalOutput")
    if debug:
        dbg_qkvT = nc.dram_tensor("dbg_qkvT", [128, 6 * S], BF16, kind="ExternalOutput")
        dbg_vext = nc.dram_tensor("dbg_vext", [128, N_KCH * VEXT_W], BF16, kind="ExternalOutput")
        dbg_att = nc.dram_tensor("dbg_att", [128, 2 * S], BF16, kind="ExternalOutput")
        dbg_den = nc.dram_tensor("dbg_den", [128, S], F32, kind="ExternalOutput")
        dbg_arena = nc.dram_tensor("dbg_arena", [128, 64 * 512], BF16, kind="ExternalOutput")

    with tile.TileContext(nc) as tc:
        with (
            tc.tile_pool(name="consts", bufs=1) as consts,
            tc.tile_pool(name="persist", bufs=1) as persist,
            tc.tile_pool(name="xs", bufs=2) as xs,
            tc.tile_pool(name="rr", bufs=2) as rrp,
            tc.tile_pool(name="osb", bufs=3) as osbp,
            tc.tile_pool(name="vtr", bufs=4) as vtrp,
            tc.tile_pool(name="qkv_ps", bufs=2, space="PSUM") as qkv_ps,
            tc.tile_pool(name="sc_ps", bufs=2, space="PSUM") as sc_ps,
            tc.tile_pool(name="avout", bufs=2, space="PSUM") as avout,
            tc.tile_pool(name="dram", bufs=1, space="DRAM") as dramp,
        ):
            recip_dram = dramp.tile([4, S], BF16, name="recip_dram")
            # ---- constants ----
            wq_sb = []
            for d in range(8):
                w = consts.tile([128, LOCAL_COLS], BF16, name=f"wq_{d}")
                nc.sync.dma_start(w[:], wqkv[d * 128 : (d + 1) * 128, :])
                wq_sb.append(w)
            wout_sb = consts.tile([128, 2 * D], BF16)
            for ch in range(2):
                nc.sync.dma_start(
                    wout_sb[:, ch * D : (ch + 1) * D],
                    wout[ch * 128 : (ch + 1) * 128, :],
                )
            bqkv_sb = consts.tile([128, 6], F32)
            nc.sync.dma_start(bqkv_sb[:], bqkv_pc[:])
            kbias_sb = consts.tile([128, N_KCH], F32)
            nc.sync.dma_start(kbias_sb[:], kbias[:])
            qmask_sb = consts.tile([128, S], BF16)
            nc.sync.dma_start(qmask_sb[:], qmask_rep[:])
            tri_sb = consts.tile([128, 128], F32)
            nc.sync.dma_start(tri_sb[:], tri[:])

            # ---- persistent state ----
            qkvT = persist.tile([128, 6 * S], BF16)
            v_ext = persist.tile([128, N_KCH * VEXT_W], BF16)
            att_u = persist.tile([128, 2 * S], BF16)
            den4 = persist.tile([128, S], F32)
            recip4 = persist.tile([128, S], BF16)
            arena = persist.tile([128, 64 * 512], BF16)
            scratch = persist.tile([1, 8], F32)

            def slot(h, j):
                # heads of a pair adjacent so one EXP writes both halves
                return (j * HEADS_PER_CORE + h) * 512

            xts = {}

            def emit_qkv_round(t, cc):
                ps = qkv_ps.tile([128, 512], F32, tag="qkvps", name=f"qps_{t}_{cc}")
                for d in range(8):
                    nc.tensor.matmul(
                        ps[:],
                        wq_sb[d][:, cc * 128 : (cc + 1) * 128],
                        xts[t][:, d * 512 : (d + 1) * 512],
                        start=(d == 0),
                        stop=(d == 7),
                    )
                nc.vector.tensor_scalar_add(
                    qkvT[:, cc * S + t * 512 : cc * S + (t + 1) * 512],
                    ps[:],
                    bqkv_sb[:, cc : cc + 1],
                )

            def emit_score_pair(t, p, j):
                """Scores piece for both heads of pair p vs k-chunk j, q-tile t.
                Two matmuls into one [128,1024] psum (one 512-half per head),
                a single EXP (same per-key bias) into two adjacent arena slots."""
                db = max(0, j * 128 - t * 512)
                sps = sc_ps.tile([128, 1024], F32, tag="scps", name=f"sps_{t}_{p}_{j}")
                for hh in range(2):
                    qrow = hh * 64
                    nc.tensor.matmul(
                        sps[:, hh * 512 + db : hh * 512 + 512],
                        qkvT[qrow : qrow + 64, (2 + p) * S + j * 128 : (2 + p) * S + (j + 1) * 128],
                        qkvT[qrow : qrow + 64, p * S + t * 512 + db : p * S + (t + 1) * 512],
                        start=True,
                        stop=True,
                    )
                s0 = slot(2 * p, j)
                if j >= 4 * t:
                    for hh in range(2):
                        nc.vector.tensor_add(
                            sps[:, hh * 512 + db : hh * 512 + db + 128],
                            sps[:, hh * 512 + db : hh * 512 + db + 128],
                            tri_sb[:],
                        )
                    if db > 0:
                        nc.gpsimd.memset(
                            arena[:, s0 : s0 + 1024].rearrange(
                                "p (g c) -> p g c", g=2
                            )[:, :, 0:db],
                            0.0,
                        )
                nc.scalar.activation(
                    arena[:, s0 : s0 + 1024].rearrange("p (g c) -> p g c", g=2)[
                        :, :, db:512
                    ],
                    sps[:].rearrange("p (g c) -> p g c", g=2)[:, :, db:512],
                    AF.Exp,
                    bias=kbias_sb[:, j : j + 1],
                    scale=SCALE,
                )

            def emit_av_chunk(t, h, j, aps, last):
                nc.tensor.matmul(
                    aps[:],
                    v_ext[:, j * VEXT_W + h * (HD + 1) : j * VEXT_W + (h + 1) * (HD + 1)],
                    arena[:, slot(h, j) : slot(h, j) + 512],
                    start=(j == 0),
                    stop=last,
                )

            def emit_head_tail(t, p, hh, aps):
                h = 2 * p + hh
                qrow = hh * 64
                nc.vector.tensor_copy(
                    den4[32 * h : 32 * h + 1, t * 512 : (t + 1) * 512], aps[64:65, :]
                )
                nc.vector.tensor_copy(
                    att_u[qrow : qrow + 64, p * S + t * 512 : p * S + (t + 1) * 512],
                    aps[0:64, :],
                )

            rr_tiles = {}

            def emit_norm(t):
                _act_recip(
                    nc,
                    recip4[:, t * 512 : (t + 1) * 512],
                    den4[:, t * 512 : (t + 1) * 512],
                    bias=EPS,
                )
                nc.vector.tensor_mul(
                    recip4[:, t * 512 : (t + 1) * 512],
                    recip4[:, t * 512 : (t + 1) * 512],
                    qmask_sb[:, t * 512 : (t + 1) * 512],
                )
                nc.scalar.dma_start(
                    recip_dram[:, t * 512 : (t + 1) * 512],
                    recip4[:, t * 512 : (t + 1) * 512]
                    .rearrange("(a b) c -> a b c", b=32)[:, 0:1, :]
                    .rearrange("a b c -> (a b) c"),
                )
                for p in range(2):
                    rr = rrp.tile([128, 512], BF16, tag="rr", name=f"rr_{t}_{p}")
                    for hh in range(2):
                        h = 2 * p + hh
                        nc.scalar.dma_start(
                            rr[hh * 64 : (hh + 1) * 64, :],
                            recip_dram[h : h + 1, t * 512 : (t + 1) * 512]
                            .to_broadcast((64, 512)),
                        )
                    rr_tiles[(t, p)] = rr

            def emit_outproj(t):
                for p in range(2):
                    rr = rr_tiles.pop((t, p))
                    nc.vector.tensor_mul(
                        att_u[:, p * S + t * 512 : p * S + (t + 1) * 512],
                        att_u[:, p * S + t * 512 : p * S + (t + 1) * 512],
                        rr[:],
                    )
                for st in range(4 * t, 4 * t + 4):
                    for n in range(2):
                        ops = avout.tile(
                            [128, 512], F32, tag="avps", name=f"ops_{st}_{n}"
                        )
                        for ch in range(2):
                            nc.tensor.matmul(
                                ops[:],
                                att_u[:, ch * S + st * 128 : ch * S + (st + 1) * 128],
                                wout_sb[:, ch * D + n * 512 : ch * D + (n + 1) * 512],
                                start=(ch == 0),
                                stop=(ch == 1),
                            )
                        ob = osbp.tile([128, 512], BF16, tag="osb", name=f"ob_{st}_{n}")
                        nc.vector.tensor_copy(ob[:], ops[:])
                        nc.sync.dma_start(
                            out[st * 128 : (st + 1) * 128, n * 512 : (n + 1) * 512],
                            ob[:],
                        )

            # ==================== main wavefront loop ====================
            first = True
            for t in range(4):
                n_j = 4 * t + 4
                xt = xs.tile([128, 8 * 512], BF16, tag="xs", name=f"xs_{t}")
                for dg in range(2):
                    nc.gpsimd.dma_start(
                        xt[:, dg * 2048 : (dg + 1) * 2048].rearrange(
                            "p (d c) -> p d c", d=4
                        ),
                        xT[dg * 512 : (dg + 1) * 512, t * 512 : (t + 1) * 512]
                        .rearrange("(d p) c -> p d c", p=128),
                    )
                xts[t] = xt
                if first:
                    # engine-side memsets after the first DMA triggers
                    nc.gpsimd.memset(den4[:], 1.0)
                    nc.gpsimd.memset(scratch[:], 1.0)
                    first = False

                emit_qkv_round(t, 0)
                emit_qkv_round(t, 1)
                if t > 0:
                    emit_outproj(t - 1)
                emit_qkv_round(t, 4)
                emit_qkv_round(t, 5)
                # V transposes early: XBAR DMA into 32B-aligned scratch, then
                # DVE copy into the 65-col layout; sync engine runs these
                # while the PE chews on scores
                for sc in range(4 * t, n_j):
                    base = sc * VEXT_W
                    nc.gpsimd.memset(
                        v_ext[:, base : base + VEXT_W].rearrange(
                            "p (h c) -> p h c", h=HEADS_PER_CORE
                        )[:, :, HD : HD + 1],
                        1.0,
                    )
                    for hp in range(2):
                        vt = vtrp.tile([128, 128], BF16, tag="vtr", name=f"vt_{sc}_{hp}")
                        nc.sync.dma_start_transpose(
                            vt[:],
                            qkvT[:, (4 + hp) * S + sc * 128 : (4 + hp) * S + (sc + 1) * 128],
                        )
                        nc.vector.tensor_copy(
                            v_ext[:, base + hp * 130 : base + hp * 130 + 130]
                            .rearrange("p (g c) -> p g c", g=2)[:, :, 0:HD],
                            vt[:].rearrange("p (g c) -> p g c", g=2),
                        )

                # per pair: scores (paired-head pieces) interleaved with AV
                # chunks (lag 2); diag scores need k(t) so qkv rounds 2,3 are
                # emitted just before pair 0 reaches them
                for p in range(2):
                    aps = [
                        avout.tile(
                            [65, 512], F32, tag="avps", padded_shape=[128, 512],
                            name=f"aps_{t}_{2 * p + hh}",
                        )
                        for hh in range(2)
                    ]

                    def avpair(j, last):
                        for hh in range(2):
                            emit_av_chunk(t, 2 * p + hh, j, aps[hh], last)

                    LAG = 4
                    for j in range(0, 4 * t):
                        emit_score_pair(t, p, j)
                        if j >= LAG:
                            avpair(j - LAG, False)
                    if p == 0:
                        emit_qkv_round(t, 2)
                        emit_qkv_round(t, 3)
                    for j in range(4 * t, n_j):
                        emit_score_pair(t, p, j)
                        if j >= LAG:
                            avpair(j - LAG, False)
                    for j in range(max(0, n_j - LAG), n_j):
                        avpair(j, j == n_j - 1)
                    for hh in range(2):
                        emit_head_tail(t, p, hh, aps[hh])

                # preload the reciprocal ACT table off the critical path
                _act_recip(nc, scratch[0:1, 0:1], scratch[0:1, 4:5], bias=EPS)
                if debug and t == 3:
                    # dump arena before norm (holds wavefront-3 P pieces)
                    nc.sync.dma_start(dbg_arena[:], arena[:])
                    nc.sync.dma_start(dbg_att[:], att_u[:])
                emit_norm(t)
            emit_outproj(3)
            if debug:
                nc.sync.dma_start(dbg_qkvT[:], qkvT[:])
                nc.sync.dma_start(dbg_vext[:], v_ext[:])
                nc.sync.dma_start(dbg_den[:], den4[:])

    return _split_waits(nc) if split_waits else nc


def make_in_maps(x, attention_mask, Wqkv, bqkv, Wout):
    """Shard full inputs into the 8 per-core input dicts."""
    import ml_dtypes

    BF = ml_dtypes.bfloat16
    x = np.asarray(x, np.float32)
    attention_mask = np.asarray(attention_mask)
    Wqkv = np.asarray(Wqkv, np.float32)
    bqkv = np.asarray(bqkv, np.float32)
    Wout = np.asarray(Wout, np.float32)

    tri = np.where(
        np.arange(128)[:, None] <= np.arange(128)[None, :], 0.0, NEG
    ).astype(np.float32)

    in_maps = []
    for c in range(CORES):
        b, g = divmod(c, 4)
        cs = 256 * g  # local col start within each of q/k/v blocks
        wq = Wqkv[:, cs : cs + 256]
        wk = Wqkv[:, D + cs : D + cs + 256]
        wv = Wqkv[:, 2 * D + cs : 2 * D + cs + 256]
        w_local = np.ascontiguousarray(np.concatenate([wq, wk, wv], axis=1))
        b_local = np.concatenate(
            [bqkv[cs : cs + 256], bqkv[D + cs : D + cs + 256], bqkv[2 * D + cs : 2 * D + cs + 256]]
        )
        bqkv_pc = np.ascontiguousarray(b_local.reshape(6, 128).T)
        wout_l = np.ascontiguousarray(Wout[cs : cs + 256, :])
        m = attention_mask[b].astype(np.float32)
        kb = np.where(m > 0, 0.0, NEG).astype(np.float32)
        kbias_pc = np.ascontiguousarray(kb.reshape(N_KCH, 128).T)
        qmask_rep = np.ascontiguousarray(
            np.broadcast_to(m[None, :], (128, S)).astype(BF)
        )
        in_maps.append(
            {
                "xT": np.ascontiguousarray(x[b].T).astype(BF),
                "wqkv": w_local.astype(BF),
                "bqkv_pc": bqkv_pc,
                "wout": wout_l.astype(BF),
                "kbias": kbias_pc,
                "qmask_rep": qmask_rep,
                "tri": tri,
            }
        )
    return in_maps


_NC_CACHE = {}


def _get_nc():
    if "nc" not in _NC_CACHE:
        _NC_CACHE["nc"] = build_nc()
    return _NC_CACHE["nc"]


def kernel(x, attention_mask, Wqkv, bqkv, Wout, bout, _trace=False, _trace_kwargs=None):
    bout = np.asarray(bout, np.float32)
    in_maps = make_in_maps(x, attention_mask, Wqkv, bqkv, Wout)
    nc = _get_nc()
    res = run_bass_kernel_spmd(
        nc,
        in_maps,
        list(range(CORES)),
        trace=_trace,
        **(_trace_kwargs or {}),
    )
    outs = [np.asarray(res.results[c]["out"], np.float32) for c in range(CORES)]
    full = np.empty((B, S, D), np.float32)
    for b in range(B):
        full[b] = outs[4 * b] + outs[4 * b + 1] + outs[4 * b + 2] + outs[4 * b + 3] + bout
    if _trace:
        return full, res
    return full


# revision 24
# speedup vs baseline: 1.5068x; 1.1000x over previous
"""Trainium2 Bass kernel for causal self-attention (B=2, S=2048, D=1024, H=16).

Sharding: 8 cores = 2 batch groups x 4 head-groups (tensor parallel).
Core c handles batch b = c // 4 and heads [4*(c%4), 4*(c%4)+4).
Each core computes a partial out-projection [S, D] in bf16; the host sums the
4 partials of each batch group (row-parallel TP unshard) and adds bout.

v2 design (wavefront): all dtypes bf16 on device (fp32 PSUM accumulate).
One wavefront per 512-wide q-tile t:
  1. qkvT[:, t] = Wqkv_local.T @ x.T in 6 single-psum-bank rounds (cc), with
     per-d-chunk weight tiles so the first matmul starts as soon as chunk 0
     and the first x tile land.
  2. scores for q-tile t against all k-chunks j <= 4t+3, exact-start pieces
     (stream [max(512t,128j), 512(t+1)) only).  P = exp(scale*s + kbias) into
     a 64-slot SBUF arena, slot (h, j), rewritten every wavefront.
  3. AV chunks interleaved with scores per head (lag 2) so the ACT engine
     (EXP) keeps pace with the PE; V_ext rows come from DMA-transposes
     (XBAR) of the v chunks - no PE transposes.
  4. denominators via the ones-column of V_ext; one batched ACT reciprocal
     per tile (recip table preloaded by a dummy op), qmask folded in with one
     DVE multiply, SBUF->SBUF broadcast DMA, normalize, out-project.  The
     norm chain of tile t is hidden under wavefront t+1's QKV rounds.
"""

import os
import sys

import numpy as np

for _p in ("/opt/trn_rl_repo",):
    if _p not in sys.path and os.path.isdir(_p):
        sys.path.insert(0, _p)

import concourse.bass as bass
import concourse.mybir as mybir
from concourse import tile
from concourse.bass_utils import run_bass_kernel_spmd

B, S, D, H = 2, 2048, 1024, 16
HD = D // H  # 64
HEADS_PER_CORE = 4
CORES = 8
LOCAL_COLS = 3 * HEADS_PER_CORE * HD  # 768 (q|k|v for 4 heads)
NEG = -1.0e30
EPS = 1.0e-9

F32 = mybir.dt.float32
BF16 = mybir.dt.bfloat16

AF = mybir.ActivationFunctionType

N_KCH = S // 128  # 16 k-chunks
VEXT_W = HEADS_PER_CORE * (HD + 1)  # 260
SCALE = float(HD) ** -0.5


def _split_waits(nc, cap=1):
    """Walrus in this container allows few sync-waits per instruction.
    Hoist excess waits onto preceding same-engine NoOps (same sequencer,
    program order => semantics preserved).  fp32-path Matmult lowers to
    LDW+MM whose LW struct takes no waits at all -> cap 0."""
    uid = [0]
    for fn in nc.m.functions:
        for bb in fn.blocks:
            insts = bb.instructions
            out = []
            for ins in insts:
                icap = 0 if isinstance(ins, mybir.InstMatmult) else cap
                si = ins.sync_info
                waits = list(si.on_wait) if (si and si.on_wait) else []
                if len(waits) > icap:
                    extra = waits[:-icap] if icap else waits
                    keep = waits[-icap:] if icap else []
                    gcap = max(cap, 1)
                    for i in range(0, len(extra), gcap):
                        grp = extra[i : i + gcap]
                        nop = mybir.InstNoOp(
                            name=f"wsplit-{uid[0]}", ins=[], outs=[]
                        )
                        uid[0] += 1
                        nop.engine = ins.engine
                        nop.sync_info = mybir.SyncInfo(on_wait=grp, on_update=[])
                        out.append(nop)
                    si.on_wait = keep
                out.append(ins)
            if len(out) != len(insts):
                insts[:] = out
    return nc


def _act_recip(nc, out_ap, in_ap, bias=0.0):
    """ACT-engine reciprocal (bass blocks ActivationFunctionType.Reciprocal
    behind an accuracy warning; ~1e-5 rel err is fine for this kernel).
    Computes 1/(x + bias)."""
    eng = nc.scalar
    inputs = [eng.lower_ap(in_ap)]
    for v in (bias, 1.0, 0.0):  # bias, scale, alpha
        inputs.append(mybir.ImmediateValue(dtype=mybir.dt.float32, value=v))
    return eng.add_instruction(
        mybir.InstActivation(
            name=eng.bass.get_next_instruction_name(),
            func=mybir.ActivationFunctionType.Reciprocal,
            ins=inputs,
            outs=[eng.lower_ap(out_ap)],
        )
    )


def build_nc(split_waits=True, debug=False):
    """Build the SPMD single-core program (same program on all 8 cores)."""
    nc = bass.Bass()

    xT = nc.dram_tensor("xT", [D, S], BF16, kind="ExternalInput")
    wqkv = nc.dram_tensor("wqkv", [128, 6 * 1024], BF16, kind="ExternalInput")
    bqkv_pc = nc.dram_tensor("bqkv_pc", [128, 6], F32, kind="ExternalInput")
    wout = nc.dram_tensor("wout", [256, D], BF16, kind="ExternalInput")
    kbias = nc.dram_tensor("kbias", [128, N_KCH], F32, kind="ExternalInput")
    qmask_rep = nc.dram_tensor("qmask_rep", [128, S], F32, kind="ExternalInput")
    tri = nc.dram_tensor("tri", [128, 128], F32, kind="ExternalInput")
    out = nc.dram_tensor("out", [S, D], BF16, kind="ExternalOutput")
    if debug:
        dbg_qkvT = nc.dram_tensor("dbg_qkvT", [128, 6 * S], BF16, kind="ExternalOutput")
        dbg_vext = nc.dram_tensor("dbg_vext", [128, N_KCH * VEXT_W], BF16, kind="ExternalOutput")
        dbg_att = nc.dram_tensor("dbg_att", [128, 2 * S], BF16, kind="ExternalOutput")
        dbg_den = nc.dram_tensor("dbg_den", [128, S], F32, kind="ExternalOutput")
        dbg_arena = nc.dram_tensor("dbg_arena", [128, 64 * 512], BF16, kind="ExternalOutput")

    with tile.TileContext(nc) as tc:
        with (
            tc.tile_pool(name="consts", bufs=1) as consts,
            tc.tile_pool(name="persist", bufs=1) as persist,
            tc.tile_pool(name="xs", bufs=2) as xs,
            tc.tile_pool(name="osb", bufs=3) as osbp,
            tc.tile_pool(name="vtr", bufs=4) as vtrp,
            tc.tile_pool(name="qkv_ps", bufs=2, space="PSUM") as qkv_ps,
            tc.tile_pool(name="sc_ps", bufs=2, space="PSUM") as sc_ps,
            tc.tile_pool(name="avout", bufs=2, space="PSUM") as avout,
        ):
            # ---- constants ----
            # cc-major packed weights: one DMA feeds an entire qkv round
            wq_sb = consts.tile([128, 6 * 1024], BF16, name="wq_sb")
            for cc in range(6):
                nc.sync.dma_start(
                    wq_sb[:, cc * 1024 : (cc + 1) * 1024],
                    wqkv[:, cc * 1024 : (cc + 1) * 1024],
                )
            wout_sb = consts.tile([128, 2 * D], BF16)
            for ch in range(2):
                nc.sync.dma_start(
                    wout_sb[:, ch * D : (ch + 1) * D],
                    wout[ch * 128 : (ch + 1) * 128, :],
                )
            bqkv_sb = consts.tile([128, 6], F32)
            nc.sync.dma_start(bqkv_sb[:], bqkv_pc[:])
            kbias_sb = consts.tile([128, N_KCH], F32)
            nc.sync.dma_start(kbias_sb[:], kbias[:])
            qmask_sb = consts.tile([128, S], F32)
            nc.sync.dma_start(qmask_sb[:], qmask_rep[:])
            ones64 = consts.tile([128, HD], BF16)
            tri_sb = consts.tile([128, 128], F32)
            nc.sync.dma_start(tri_sb[:], tri[:])

            # ---- persistent state ----
            qkvT = persist.tile([128, 6 * S], BF16)
            v_ext = persist.tile([128, N_KCH * VEXT_W], BF16)
            att_u = persist.tile([128, 2 * S], BF16)
            den4 = [persist.tile([128, S], F32, name=f"den4_{p}") for p in range(2)]
            recip4 = [persist.tile([128, S], BF16, name=f"recip4_{p}") for p in range(2)]
            arena = persist.tile([128, 64 * 512], BF16)
            scratch = persist.tile([1, 8], F32)

            def slot(h, j):
                # heads of a pair adjacent so one EXP writes both halves
                return (j * HEADS_PER_CORE + h) * 512

            xts = {}

            def emit_qkv_round(t, cc):
                ps = qkv_ps.tile([128, 512], F32, tag="qkvps", name=f"qps_{t}_{cc}")
                for d in range(8):
                    nc.tensor.matmul(
                        ps[:],
                        wq_sb[:, cc * 1024 + d * 128 : cc * 1024 + (d + 1) * 128],
                        xts[t][:, d * 512 : (d + 1) * 512],
                        start=(d == 0),
                        stop=(d == 7),
                    )
                nc.vector.tensor_scalar_add(
                    qkvT[:, cc * S + t * 512 : cc * S + (t + 1) * 512],
                    ps[:],
                    bqkv_sb[:, cc : cc + 1],
                )

            def emit_score_pair(t, p, j):
                """Scores piece for both heads of pair p vs k-chunk j, q-tile t.
                Two matmuls into one [128,1024] psum (one 512-half per head),
                a single EXP (same per-key bias) into two adjacent arena slots."""
                db = max(0, j * 128 - t * 512)
                sps = sc_ps.tile([128, 1024], F32, tag="scps", name=f"sps_{t}_{p}_{j}")
                for hh in range(2):
                    qrow = hh * 64
                    nc.tensor.matmul(
                        sps[:, hh * 512 + db : hh * 512 + 512],
                        qkvT[qrow : qrow + 64, (2 + p) * S + j * 128 : (2 + p) * S + (j + 1) * 128],
                        qkvT[qrow : qrow + 64, p * S + t * 512 + db : p * S + (t + 1) * 512],
                        start=True,
                        stop=True,
                    )
                s0 = slot(2 * p, j)
                if j >= 4 * t:
                    for hh in range(2):
                        nc.vector.tensor_add(
                            sps[:, hh * 512 + db : hh * 512 + db + 128],
                            sps[:, hh * 512 + db : hh * 512 + db + 128],
                            tri_sb[:],
                        )
                    if db > 0:
                        nc.gpsimd.memset(
                            arena[:, s0 : s0 + 1024].rearrange(
                                "p (g c) -> p g c", g=2
                            )[:, :, 0:db],
                            0.0,
                        )
                if db == 0:
                    nc.scalar.activation(
                        arena[:, s0 : s0 + 1024],
                        sps[:],
                        AF.Exp,
                        bias=kbias_sb[:, j : j + 1],
                        scale=SCALE,
                    )
                else:
                    nc.scalar.activation(
                        arena[:, s0 : s0 + 1024].rearrange("p (g c) -> p g c", g=2)[
                            :, :, db:512
                        ],
                        sps[:].rearrange("p (g c) -> p g c", g=2)[:, :, db:512],
                        AF.Exp,
                        bias=kbias_sb[:, j : j + 1],
                        scale=SCALE,
                    )

            def emit_av_chunk(t, h, j, aps, last):
                nc.tensor.matmul(
                    aps[:],
                    v_ext[:, j * VEXT_W + h * (HD + 1) : j * VEXT_W + (h + 1) * (HD + 1)],
                    arena[:, slot(h, j) : slot(h, j) + 512],
                    start=(j == 0),
                    stop=last,
                )

            def emit_head_tail(t, p, hh, aps):
                h = 2 * p + hh
                qrow = hh * 64
                nc.vector.tensor_add(
                    den4[p][32 * hh : 32 * hh + 1, t * 512 : (t + 1) * 512],
                    aps[64:65, :],
                    qmask_sb[64:65, t * 512 : (t + 1) * 512],
                )
                nc.vector.tensor_copy(
                    att_u[qrow : qrow + 64, p * S + t * 512 : p * S + (t + 1) * 512],
                    aps[0:64, :],
                )

            rr_tiles = {}

            def emit_norm(t):
                for p in range(2):
                    _act_recip(
                        nc,
                        recip4[p][:, t * 512 : (t + 1) * 512],
                        den4[p][:, t * 512 : (t + 1) * 512],
                        bias=EPS,
                    )
                rrps = sc_ps.tile([128, 1024], F32, tag="scps", name=f"rrps_{t}")
                for p in range(2):
                    for hh in range(2):
                        nc.tensor.matmul(
                            rrps[hh * 64 : (hh + 1) * 64, p * 512 : (p + 1) * 512],
                            ones64[32 * hh : 32 * hh + 1, :],
                            recip4[p][32 * hh : 32 * hh + 1, t * 512 : (t + 1) * 512],
                            start=True,
                            stop=True,
                        )
                rr_tiles[t] = rrps

            def emit_outproj(t):
                rrps = rr_tiles.pop(t)
                for p in range(2):
                    nc.vector.tensor_mul(
                        att_u[:, p * S + t * 512 : p * S + (t + 1) * 512],
                        att_u[:, p * S + t * 512 : p * S + (t + 1) * 512],
                        rrps[:, p * 512 : (p + 1) * 512],
                    )
                for st in range(4 * t, 4 * t + 4):
                    for n in range(2):
                        ops = avout.tile(
                            [128, 512], F32, tag="avps", name=f"ops_{st}_{n}"
                        )
                        for ch in range(2):
                            nc.tensor.matmul(
                                ops[:],
                                att_u[:, ch * S + st * 128 : ch * S + (st + 1) * 128],
                                wout_sb[:, ch * D + n * 512 : ch * D + (n + 1) * 512],
                                start=(ch == 0),
                                stop=(ch == 1),
                            )
                        ob = osbp.tile([128, 512], BF16, tag="osb", name=f"ob_{st}_{n}")
                        nc.vector.tensor_copy(ob[:], ops[:])
                        eng = nc.sync if (st + n) % 2 == 0 else nc.scalar
                        eng.dma_start(
                            out[st * 128 : (st + 1) * 128, n * 512 : (n + 1) * 512],
                            ob[:],
                        )

            # ==================== main wavefront loop ====================
            first = True
            for t in range(4):
                n_j = 4 * t + 4
                xt = xs.tile([128, 8 * 512], BF16, tag="xs", name=f"xs_{t}")
                groups = ((0, 1), (1, 4), (4, 8)) if first else ((0, 4), (4, 8))
                for d0, d1 in groups:
                    nc.gpsimd.dma_start(
                        xt[:, d0 * 512 : d1 * 512].rearrange(
                            "p (d c) -> p d c", d=d1 - d0
                        ),
                        xT[d0 * 128 : d1 * 128, t * 512 : (t + 1) * 512]
                        .rearrange("(d p) c -> p d c", p=128),
                    )
                xts[t] = xt
                if first:
                    # engine-side memsets after the first DMA triggers
                    nc.gpsimd.memset(den4[0][:], 1.0)
                    nc.gpsimd.memset(den4[1][:], 1.0)
                    nc.gpsimd.memset(scratch[:], 1.0)
                    nc.gpsimd.memset(ones64[:], 1.0)
                    first = False

                emit_qkv_round(t, 0)
                emit_qkv_round(t, 1)
                emit_qkv_round(t, 4)
                emit_qkv_round(t, 5)
                # V transposes early: XBAR DMA into 32B-aligned scratch, then
                # DVE copy into the 65-col layout; sync engine runs these
                # while the PE chews on scores
                for sc in range(4 * t, n_j):
                    base = sc * VEXT_W
                    nc.gpsimd.memset(
                        v_ext[:, base : base + VEXT_W].rearrange(
                            "p (h c) -> p h c", h=HEADS_PER_CORE
                        )[:, :, HD : HD + 1],
                        1.0,
                    )
                    for hp in range(2):
                        vt = vtrp.tile([128, 128], BF16, tag="vtr", name=f"vt_{sc}_{hp}")
                        nc.sync.dma_start_transpose(
                            vt[:],
                            qkvT[:, (4 + hp) * S + sc * 128 : (4 + hp) * S + (sc + 1) * 128],
                        )
                        nc.vector.tensor_copy(
                            v_ext[:, base + hp * 130 : base + hp * 130 + 130]
                            .rearrange("p (g c) -> p g c", g=2)[:, :, 0:HD],
                            vt[:].rearrange("p (g c) -> p g c", g=2),
                        )

                if t > 0:
                    emit_outproj(t - 1)

                # per pair: scores (paired-head pieces) interleaved with AV
                # chunks (lag 2); diag scores need k(t) so qkv rounds 2,3 are
                # emitted just before pair 0 reaches them
                for p in range(2):
                    aps = [
                        avout.tile(
                            [65, 512], F32, tag="avps", padded_shape=[128, 512],
                            name=f"aps_{t}_{2 * p + hh}",
                        )
                        for hh in range(2)
                    ]

                    def avpair(j, last):
                        for hh in range(2):
                            emit_av_chunk(t, 2 * p + hh, j, aps[hh], last)

                    LAG = 4
                    for j in range(0, 4 * t):
                        emit_score_pair(t, p, j)
                        if j >= LAG:
                            avpair(j - LAG, False)
                    if p == 0:
                        emit_qkv_round(t, 2)
                        emit_qkv_round(t, 3)
                    for j in range(4 * t, n_j):
                        emit_score_pair(t, p, j)
                        if j >= LAG:
                            avpair(j - LAG, False)
                    for j in range(max(0, n_j - LAG), n_j):
                        avpair(j, j == n_j - 1)
                    for hh in range(2):
                        emit_head_tail(t, p, hh, aps[hh])

                # preload the reciprocal ACT table off the critical path
                _act_recip(nc, scratch[0:1, 0:1], scratch[0:1, 4:5], bias=EPS)
                if debug and t == 3:
                    # dump arena before norm (holds wavefront-3 P pieces)
                    nc.sync.dma_start(dbg_arena[:], arena[:])
                    nc.sync.dma_start(dbg_att[:], att_u[:])
                emit_norm(t)
            emit_outproj(3)
            if debug:
                nc.sync.dma_start(dbg_qkvT[:], qkvT[:])
                nc.sync.dma_start(dbg_vext[:], v_ext[:])
                nc.sync.dma_start(dbg_den[:], den4[0][:])

    return _split_waits(nc) if split_waits else nc


def make_in_maps(x, attention_mask, Wqkv, bqkv, Wout):
    """Shard full inputs into the 8 per-core input dicts."""
    import ml_dtypes

    BF = ml_dtypes.bfloat16
    x = np.asarray(x, np.float32)
    attention_mask = np.asarray(attention_mask)
    Wqkv = np.asarray(Wqkv, np.float32)
    bqkv = np.asarray(bqkv, np.float32)
    Wout = np.asarray(Wout, np.float32)

    tri = np.where(
        np.arange(128)[:, None] <= np.arange(128)[None, :], 0.0, NEG
    ).astype(np.float32)

    in_maps = []
    for c in range(CORES):
        b, g = divmod(c, 4)
        cs = 256 * g  # local col start within each of q/k/v blocks
        wq = Wqkv[:, cs : cs + 256]
        wk = Wqkv[:, D + cs : D + cs + 256]
        wv = Wqkv[:, 2 * D + cs : 2 * D + cs + 256]
        w_local = np.concatenate([wq, wk, wv], axis=1)  # [1024, 768]
        # cc-major pack: packed[p, cc*1024 + d*128 + c] = w_local[d*128+p, cc*128+c]
        w_packed = np.ascontiguousarray(
            w_local.reshape(8, 128, 6, 128).transpose(1, 2, 0, 3).reshape(128, 6 * 1024)
        )
        b_local = np.concatenate(
            [bqkv[cs : cs + 256], bqkv[D + cs : D + cs + 256], bqkv[2 * D + cs : 2 * D + cs + 256]]
        )
        bqkv_pc = np.ascontiguousarray(b_local.reshape(6, 128).T)
        wout_l = np.ascontiguousarray(Wout[cs : cs + 256, :])
        m = attention_mask[b].astype(np.float32)
        kb = np.where(m > 0, 0.0, NEG).astype(np.float32)
        kbias_pc = np.ascontiguousarray(kb.reshape(N_KCH, 128).T)
        qhuge = np.where(m > 0, 0.0, float(2.0 ** 40)).astype(np.float32)
        qmask_rep = np.ascontiguousarray(np.broadcast_to(qhuge[None, :], (128, S)))
        in_maps.append(
            {
                "xT": np.ascontiguousarray(x[b].T).astype(BF),
                "wqkv": w_packed.astype(BF),
                "bqkv_pc": bqkv_pc,
                "wout": wout_l.astype(BF),
                "kbias": kbias_pc,
                "qmask_rep": qmask_rep,
                "tri": tri,
            }
        )
    return in_maps


_NC_CACHE = {}


def _get_nc():
    if "nc" not in _NC_CACHE:
        _NC_CACHE["nc"] = build_nc()
    return _NC_CACHE["nc"]


def kernel(x, attention_mask, Wqkv, bqkv, Wout, bout, _trace=False, _trace_kwargs=None):
    bout = np.asarray(bout, np.float32)
    in_maps = make_in_maps(x, attention_mask, Wqkv, bqkv, Wout)
    nc = _get_nc()
    res = run_bass_kernel_spmd(
        nc,
        in_maps,
        list(range(CORES)),
        trace=_trace,
        **(_trace_kwargs or {}),
    )
    outs = [np.asarray(res.results[c]["out"], np.float32) for c in range(CORES)]
    full = np.empty((B, S, D), np.float32)
    for b in range(B):
        full[b] = outs[4 * b] + outs[4 * b + 1] + outs[4 * b + 2] + outs[4 * b + 3] + bout
    if _trace:
        return full, res
    return full


# revision 25
# speedup vs baseline: 1.6039x; 1.0644x over previous
"""Trainium2 Bass kernel for causal self-attention (B=2, S=2048, D=1024, H=16).

Sharding: 8 cores = 2 batch groups x 4 head-groups (tensor parallel).
Core c handles batch b = c // 4 and heads [4*(c%4), 4*(c%4)+4).
Each core computes a partial out-projection [S, D] in bf16; the host sums the
4 partials of each batch group (row-parallel TP unshard) and adds bout.

v2 design (wavefront): all dtypes bf16 on device (fp32 PSUM accumulate).
One wavefront per 512-wide q-tile t:
  1. qkvT[:, t] = Wqkv_local.T @ x.T in 6 single-psum-bank rounds (cc), with
     per-d-chunk weight tiles so the first matmul starts as soon as chunk 0
     and the first x tile land.
  2. scores for q-tile t against all k-chunks j <= 4t+3, exact-start pieces
     (stream [max(512t,128j), 512(t+1)) only).  P = exp(scale*s + kbias) into
     a 64-slot SBUF arena, slot (h, j), rewritten every wavefront.
  3. AV chunks interleaved with scores per head (lag 2) so the ACT engine
     (EXP) keeps pace with the PE; V_ext rows come from DMA-transposes
     (XBAR) of the v chunks - no PE transposes.
  4. denominators via the ones-column of V_ext; one batched ACT reciprocal
     per tile (recip table preloaded by a dummy op), qmask folded in with one
     DVE multiply, SBUF->SBUF broadcast DMA, normalize, out-project.  The
     norm chain of tile t is hidden under wavefront t+1's QKV rounds.
"""

import os
import sys

import numpy as np

for _p in ("/opt/trn_rl_repo",):
    if _p not in sys.path and os.path.isdir(_p):
        sys.path.insert(0, _p)

import concourse.bass as bass
import concourse.mybir as mybir
from concourse import tile
from concourse.bass_utils import run_bass_kernel_spmd

B, S, D, H = 2, 2048, 1024, 16
HD = D // H  # 64
HEADS_PER_CORE = 4
CORES = 8
LOCAL_COLS = 3 * HEADS_PER_CORE * HD  # 768 (q|k|v for 4 heads)
NEG = -1.0e30
EPS = 1.0e-9

F32 = mybir.dt.float32
BF16 = mybir.dt.bfloat16

AF = mybir.ActivationFunctionType

N_KCH = S // 128  # 16 k-chunks
VEXT_W = HEADS_PER_CORE * (HD + 1)  # 260
SCALE = float(HD) ** -0.5


def _split_waits(nc, cap=1):
    """Walrus in this container allows few sync-waits per instruction.
    Hoist excess waits onto preceding same-engine NoOps (same sequencer,
    program order => semantics preserved).  fp32-path Matmult lowers to
    LDW+MM whose LW struct takes no waits at all -> cap 0."""
    uid = [0]
    for fn in nc.m.functions:
        for bb in fn.blocks:
            insts = bb.instructions
            out = []
            for ins in insts:
                icap = 0 if isinstance(ins, mybir.InstMatmult) else cap
                si = ins.sync_info
                waits = list(si.on_wait) if (si and si.on_wait) else []
                if len(waits) > icap:
                    extra = waits[:-icap] if icap else waits
                    keep = waits[-icap:] if icap else []
                    gcap = max(cap, 1)
                    for i in range(0, len(extra), gcap):
                        grp = extra[i : i + gcap]
                        nop = mybir.InstNoOp(
                            name=f"wsplit-{uid[0]}", ins=[], outs=[]
                        )
                        uid[0] += 1
                        nop.engine = ins.engine
                        nop.sync_info = mybir.SyncInfo(on_wait=grp, on_update=[])
                        out.append(nop)
                    si.on_wait = keep
                out.append(ins)
            if len(out) != len(insts):
                insts[:] = out
    return nc


def _act_recip(nc, out_ap, in_ap, bias=0.0):
    """ACT-engine reciprocal (bass blocks ActivationFunctionType.Reciprocal
    behind an accuracy warning; ~1e-5 rel err is fine for this kernel).
    Computes 1/(x + bias)."""
    eng = nc.scalar
    inputs = [eng.lower_ap(in_ap)]
    for v in (bias, 1.0, 0.0):  # bias, scale, alpha
        inputs.append(mybir.ImmediateValue(dtype=mybir.dt.float32, value=v))
    return eng.add_instruction(
        mybir.InstActivation(
            name=eng.bass.get_next_instruction_name(),
            func=mybir.ActivationFunctionType.Reciprocal,
            ins=inputs,
            outs=[eng.lower_ap(out_ap)],
        )
    )


def build_nc(split_waits=True, debug=False):
    """Build the SPMD single-core program (same program on all 8 cores)."""
    nc = bass.Bass()

    xT = nc.dram_tensor("xT", [D, S], BF16, kind="ExternalInput")
    wqkv = nc.dram_tensor("wqkv", [128, 6 * 1024], BF16, kind="ExternalInput")
    bqkv_pc = nc.dram_tensor("bqkv_pc", [128, 6], F32, kind="ExternalInput")
    wout = nc.dram_tensor("wout", [256, D], BF16, kind="ExternalInput")
    kbias = nc.dram_tensor("kbias", [128, N_KCH], F32, kind="ExternalInput")
    qmask_rep = nc.dram_tensor("qmask_rep", [128, S], F32, kind="ExternalInput")
    tri = nc.dram_tensor("tri", [128, 128], F32, kind="ExternalInput")
    out = nc.dram_tensor("out", [S, D], BF16, kind="ExternalOutput")
    if debug:
        dbg_qkvT = nc.dram_tensor("dbg_qkvT", [128, 6 * S], BF16, kind="ExternalOutput")
        dbg_vext = nc.dram_tensor("dbg_vext", [128, N_KCH * VEXT_W], BF16, kind="ExternalOutput")
        dbg_att = nc.dram_tensor("dbg_att", [128, 2 * S], BF16, kind="ExternalOutput")
        dbg_den = nc.dram_tensor("dbg_den", [128, S], F32, kind="ExternalOutput")
        dbg_arena = nc.dram_tensor("dbg_arena", [128, 64 * 512], BF16, kind="ExternalOutput")

    with tile.TileContext(nc) as tc:
        with (
            tc.tile_pool(name="consts", bufs=1) as consts,
            tc.tile_pool(name="persist", bufs=1) as persist,
            tc.tile_pool(name="xs", bufs=2) as xs,
            tc.tile_pool(name="osb", bufs=8) as osbp,
            tc.tile_pool(name="vtr", bufs=4) as vtrp,
            tc.tile_pool(name="qkv_ps", bufs=2, space="PSUM") as qkv_ps,
            tc.tile_pool(name="sc_ps", bufs=2, space="PSUM") as sc_ps,
            tc.tile_pool(name="avout", bufs=2, space="PSUM") as avout,
        ):
            # ---- constants ----
            # cc-major packed weights: one DMA feeds an entire qkv round
            wq_sb = consts.tile([128, 6 * 1024], BF16, name="wq_sb")
            for cc in range(6):
                nc.sync.dma_start(
                    wq_sb[:, cc * 1024 : (cc + 1) * 1024],
                    wqkv[:, cc * 1024 : (cc + 1) * 1024],
                )
            wout_sb = consts.tile([128, 2 * D], BF16)
            for ch in range(2):
                nc.sync.dma_start(
                    wout_sb[:, ch * D : (ch + 1) * D],
                    wout[ch * 128 : (ch + 1) * 128, :],
                )
            bqkv_sb = consts.tile([128, 6], F32)
            nc.sync.dma_start(bqkv_sb[:], bqkv_pc[:])
            kbias_sb = consts.tile([128, N_KCH], F32)
            nc.sync.dma_start(kbias_sb[:], kbias[:])
            qmask_sb = consts.tile([128, S], F32)
            nc.sync.dma_start(qmask_sb[:], qmask_rep[:])
            ones64 = consts.tile([128, HD], BF16)
            tri_sb = consts.tile([128, 128], F32)
            nc.sync.dma_start(tri_sb[:], tri[:])

            # ---- persistent state ----
            qkvT = persist.tile([128, 6 * S], BF16)
            v_ext = persist.tile([128, N_KCH * VEXT_W], BF16)
            att_u = persist.tile([128, 2 * S], BF16)
            den4 = [persist.tile([128, S], F32, name=f"den4_{p}") for p in range(2)]
            recip4 = [persist.tile([128, S], BF16, name=f"recip4_{p}") for p in range(2)]
            arena = persist.tile([128, 64 * 512], BF16)
            scratch = persist.tile([1, 8], F32)

            def slot(h, j):
                # heads of a pair adjacent so one EXP writes both halves
                return (j * HEADS_PER_CORE + h) * 512

            xts = {}

            def emit_qkv_round(t, cc):
                ps = qkv_ps.tile([128, 512], F32, tag="qkvps", name=f"qps_{t}_{cc}")
                for d in range(8):
                    nc.tensor.matmul(
                        ps[:],
                        wq_sb[:, cc * 1024 + d * 128 : cc * 1024 + (d + 1) * 128],
                        xts[t][:, d * 512 : (d + 1) * 512],
                        start=(d == 0),
                        stop=(d == 7),
                    )
                nc.vector.tensor_scalar_add(
                    qkvT[:, cc * S + t * 512 : cc * S + (t + 1) * 512],
                    ps[:],
                    bqkv_sb[:, cc : cc + 1],
                )

            def emit_score_pair(t, p, j):
                """Scores piece for both heads of pair p vs k-chunk j, q-tile t.
                Two matmuls into one [128,1024] psum (one 512-half per head),
                a single EXP (same per-key bias) into two adjacent arena slots."""
                db = max(0, j * 128 - t * 512)
                sps = sc_ps.tile([128, 1024], F32, tag="scps", name=f"sps_{t}_{p}_{j}")
                for hh in range(2):
                    qrow = hh * 64
                    nc.tensor.matmul(
                        sps[:, hh * 512 + db : hh * 512 + 512],
                        qkvT[qrow : qrow + 64, (2 + p) * S + j * 128 : (2 + p) * S + (j + 1) * 128],
                        qkvT[qrow : qrow + 64, p * S + t * 512 + db : p * S + (t + 1) * 512],
                        start=True,
                        stop=True,
                    )
                s0 = slot(2 * p, j)
                if j >= 4 * t:
                    for hh in range(2):
                        nc.vector.tensor_add(
                            sps[:, hh * 512 + db : hh * 512 + db + 128],
                            sps[:, hh * 512 + db : hh * 512 + db + 128],
                            tri_sb[:],
                        )
                    if db > 0:
                        nc.gpsimd.memset(
                            arena[:, s0 : s0 + 1024].rearrange(
                                "p (g c) -> p g c", g=2
                            )[:, :, 0:db],
                            0.0,
                        )
                if db == 0:
                    nc.scalar.activation(
                        arena[:, s0 : s0 + 1024],
                        sps[:],
                        AF.Exp,
                        bias=kbias_sb[:, j : j + 1],
                        scale=SCALE,
                    )
                else:
                    nc.scalar.activation(
                        arena[:, s0 : s0 + 1024].rearrange("p (g c) -> p g c", g=2)[
                            :, :, db:512
                        ],
                        sps[:].rearrange("p (g c) -> p g c", g=2)[:, :, db:512],
                        AF.Exp,
                        bias=kbias_sb[:, j : j + 1],
                        scale=SCALE,
                    )

            def emit_av_chunk(t, h, j, aps, last):
                nc.tensor.matmul(
                    aps[:],
                    v_ext[:, j * VEXT_W + h * (HD + 1) : j * VEXT_W + (h + 1) * (HD + 1)],
                    arena[:, slot(h, j) : slot(h, j) + 512],
                    start=(j == 0),
                    stop=last,
                )

            def emit_head_tail(t, p, hh, aps):
                h = 2 * p + hh
                qrow = hh * 64
                nc.vector.tensor_add(
                    den4[p][32 * hh : 32 * hh + 1, t * 512 : (t + 1) * 512],
                    aps[64:65, :],
                    qmask_sb[64:65, t * 512 : (t + 1) * 512],
                )
                nc.vector.tensor_copy(
                    att_u[qrow : qrow + 64, p * S + t * 512 : p * S + (t + 1) * 512],
                    aps[0:64, :],
                )

            rr_tiles = {}

            def emit_norm(t):
                for p in range(2):
                    _act_recip(
                        nc,
                        recip4[p][:, t * 512 : (t + 1) * 512],
                        den4[p][:, t * 512 : (t + 1) * 512],
                        bias=EPS,
                    )
                rrps = [
                    avout.tile([128, 512], F32, tag="avps", name=f"rrps_{t}_{p}")
                    for p in range(2)
                ]
                for p in range(2):
                    for hh in range(2):
                        nc.tensor.matmul(
                            rrps[p][hh * 64 : (hh + 1) * 64, :],
                            ones64[32 * hh : 32 * hh + 1, :],
                            recip4[p][32 * hh : 32 * hh + 1, t * 512 : (t + 1) * 512],
                            start=True,
                            stop=True,
                        )
                rr_tiles[t] = rrps

            def emit_outproj(t):
                rrps = rr_tiles.pop(t)
                for p in range(2):
                    nc.vector.tensor_mul(
                        att_u[:, p * S + t * 512 : p * S + (t + 1) * 512],
                        att_u[:, p * S + t * 512 : p * S + (t + 1) * 512],
                        rrps[p][:],
                    )
                for st in range(4 * t, 4 * t + 4):
                    for n in range(2):
                        ops = avout.tile(
                            [128, 512], F32, tag="avps", name=f"ops_{st}_{n}"
                        )
                        for ch in range(2):
                            nc.tensor.matmul(
                                ops[:],
                                att_u[:, ch * S + st * 128 : ch * S + (st + 1) * 128],
                                wout_sb[:, ch * D + n * 512 : ch * D + (n + 1) * 512],
                                start=(ch == 0),
                                stop=(ch == 1),
                            )
                        ob = osbp.tile([128, 512], BF16, tag="osb", name=f"ob_{st}_{n}")
                        nc.vector.tensor_copy(ob[:], ops[:])
                        nc.scalar.dma_start(
                            out[st * 128 : (st + 1) * 128, n * 512 : (n + 1) * 512],
                            ob[:],
                        )

            # ==================== main wavefront loop ====================
            first = True
            for t in range(4):
                n_j = 4 * t + 4
                xt = xs.tile([128, 8 * 512], BF16, tag="xs", name=f"xs_{t}")
                groups = ((0, 1), (1, 4), (4, 8)) if first else ((0, 4), (4, 8))
                for d0, d1 in groups:
                    nc.gpsimd.dma_start(
                        xt[:, d0 * 512 : d1 * 512].rearrange(
                            "p (d c) -> p d c", d=d1 - d0
                        ),
                        xT[d0 * 128 : d1 * 128, t * 512 : (t + 1) * 512]
                        .rearrange("(d p) c -> p d c", p=128),
                    )
                xts[t] = xt
                if first:
                    # engine-side memsets after the first DMA triggers
                    nc.gpsimd.memset(den4[0][:], 1.0)
                    nc.gpsimd.memset(den4[1][:], 1.0)
                    nc.gpsimd.memset(scratch[:], 1.0)
                    nc.gpsimd.memset(ones64[:], 1.0)
                    first = False

                emit_qkv_round(t, 0)
                emit_qkv_round(t, 1)
                emit_qkv_round(t, 4)
                emit_qkv_round(t, 5)
                # V transposes early: XBAR DMA into 32B-aligned scratch, then
                # DVE copy into the 65-col layout; sync engine runs these
                # while the PE chews on scores
                for sc in range(4 * t, n_j):
                    base = sc * VEXT_W
                    nc.gpsimd.memset(
                        v_ext[:, base : base + VEXT_W].rearrange(
                            "p (h c) -> p h c", h=HEADS_PER_CORE
                        )[:, :, HD : HD + 1],
                        1.0,
                    )
                    for hp in range(2):
                        vt = vtrp.tile([128, 128], BF16, tag="vtr", name=f"vt_{sc}_{hp}")
                        nc.sync.dma_start_transpose(
                            vt[:],
                            qkvT[:, (4 + hp) * S + sc * 128 : (4 + hp) * S + (sc + 1) * 128],
                        )
                        nc.vector.tensor_copy(
                            v_ext[:, base + hp * 130 : base + hp * 130 + 130]
                            .rearrange("p (g c) -> p g c", g=2)[:, :, 0:HD],
                            vt[:].rearrange("p (g c) -> p g c", g=2),
                        )

                if t > 0:
                    emit_outproj(t - 1)

                # per pair: scores (paired-head pieces) interleaved with AV
                # chunks (lag 2); diag scores need k(t) so qkv rounds 2,3 are
                # emitted just before pair 0 reaches them
                for p in range(2):
                    aps = [
                        avout.tile(
                            [65, 512], F32, tag="avps", padded_shape=[128, 512],
                            name=f"aps_{t}_{2 * p + hh}",
                        )
                        for hh in range(2)
                    ]

                    def avpair(j, last):
                        for hh in range(2):
                            emit_av_chunk(t, 2 * p + hh, j, aps[hh], last)

                    LAG = 4
                    for j in range(0, 4 * t):
                        emit_score_pair(t, p, j)
                        if j >= LAG:
                            avpair(j - LAG, False)
                    if p == 0:
                        emit_qkv_round(t, 2)
                        emit_qkv_round(t, 3)
                    for j in range(4 * t, n_j):
                        emit_score_pair(t, p, j)
                        if j >= LAG:
                            avpair(j - LAG, False)
                    for j in range(max(0, n_j - LAG), n_j):
                        avpair(j, j == n_j - 1)
                    for hh in range(2):
                        emit_head_tail(t, p, hh, aps[hh])

                # preload the reciprocal ACT table off the critical path
                _act_recip(nc, scratch[0:1, 0:1], scratch[0:1, 4:5], bias=EPS)
                if debug and t == 3:
                    # dump arena before norm (holds wavefront-3 P pieces)
                    nc.sync.dma_start(dbg_arena[:], arena[:])
                    nc.sync.dma_start(dbg_att[:], att_u[:])
                emit_norm(t)
            emit_outproj(3)
            if debug:
                nc.sync.dma_start(dbg_qkvT[:], qkvT[:])
                nc.sync.dma_start(dbg_vext[:], v_ext[:])
                nc.sync.dma_start(dbg_den[:], den4[0][:])

    return _split_waits(nc) if split_waits else nc


def make_in_maps(x, attention_mask, Wqkv, bqkv, Wout):
    """Shard full inputs into the 8 per-core input dicts."""
    import ml_dtypes

    BF = ml_dtypes.bfloat16
    x = np.asarray(x, np.float32)
    attention_mask = np.asarray(attention_mask)
    Wqkv = np.asarray(Wqkv, np.float32)
    bqkv = np.asarray(bqkv, np.float32)
    Wout = np.asarray(Wout, np.float32)

    tri = np.where(
        np.arange(128)[:, None] <= np.arange(128)[None, :], 0.0, NEG
    ).astype(np.float32)

    in_maps = []
    for c in range(CORES):
        b, g = divmod(c, 4)
        cs = 256 * g  # local col start within each of q/k/v blocks
        wq = Wqkv[:, cs : cs + 256]
        wk = Wqkv[:, D + cs : D + cs + 256]
        wv = Wqkv[:, 2 * D + cs : 2 * D + cs + 256]
        w_local = np.concatenate([wq, wk, wv], axis=1)  # [1024, 768]
        # cc-major pack: packed[p, cc*1024 + d*128 + c] = w_local[d*128+p, cc*128+c]
        w_packed = np.ascontiguousarray(
            w_local.reshape(8, 128, 6, 128).transpose(1, 2, 0, 3).reshape(128, 6 * 1024)
        )
        b_local = np.concatenate(
            [bqkv[cs : cs + 256], bqkv[D + cs : D + cs + 256], bqkv[2 * D + cs : 2 * D + cs + 256]]
        )
        bqkv_pc = np.ascontiguousarray(b_local.reshape(6, 128).T)
        wout_l = np.ascontiguousarray(Wout[cs : cs + 256, :])
        m = attention_mask[b].astype(np.float32)
        kb = np.where(m > 0, 0.0, NEG).astype(np.float32)
        kbias_pc = np.ascontiguousarray(kb.reshape(N_KCH, 128).T)
        qhuge = np.where(m > 0, 0.0, float(2.0 ** 40)).astype(np.float32)
        qmask_rep = np.ascontiguousarray(np.broadcast_to(qhuge[None, :], (128, S)))
        in_maps.append(
            {
                "xT": np.ascontiguousarray(x[b].T).astype(BF),
                "wqkv": w_packed.astype(BF),
                "bqkv_pc": bqkv_pc,
                "wout": wout_l.astype(BF),
                "kbias": kbias_pc,
                "qmask_rep": qmask_rep,
                "tri": tri,
            }
        )
    return in_maps


_NC_CACHE = {}


def _get_nc():
    if "nc" not in _NC_CACHE:
        _NC_CACHE["nc"] = build_nc()
    return _NC_CACHE["nc"]


def kernel(x, attention_mask, Wqkv, bqkv, Wout, bout, _trace=False, _trace_kwargs=None):
    bout = np.asarray(bout, np.float32)
    in_maps = make_in_maps(x, attention_mask, Wqkv, bqkv, Wout)
    nc = _get_nc()
    res = run_bass_kernel_spmd(
        nc,
        in_maps,
        list(range(CORES)),
        trace=_trace,
        **(_trace_kwargs or {}),
    )
    outs = [np.asarray(res.results[c]["out"], np.float32) for c in range(CORES)]
    full = np.empty((B, S, D), np.float32)
    for b in range(B):
        full[b] = outs[4 * b] + outs[4 * b + 1] + outs[4 * b + 2] + outs[4 * b + 3] + bout
    if _trace:
        return full, res
    return full
